# revision 8
# baseline (speedup 1.0000x reference)
"""Auditory spectrogram kernel for Trainium2 (8 NeuronCores, Bass/Tile).

Pipeline per the reference:
  y1 = order-4 IIR cochlear filterbank (129 channels, per-channel B/A) over wav [8, 64000]
  y2 = sigmoid(y1); y2 = 1st-order IIR (beta) over time
  y4 = relu(y2[c] - y2[c-1]); y5 = 1st-order IIR (alpha); downsample every 256 -> [8, 129, 250]

All linear recurrences are blocked-FIR matmuls on TensorE (fp16 operands,
fp32 PSUM). Channel 0's output is exactly zero, so the 128 real output
channels are sharded 16 per core; each core computes 16 channels + a
1-channel halo.

Layout: time blocked into 500 blocks of 128; partition = position in block,
free = (batch, block). Per core:
  S1  per-channel banded-Toeplitz matmuls -> psum [128, 4x512] (4 batches)
  Act one fused sigmoid per 4 batches -> s tiles fp16
  S2  hair-cell LPF: T0/T1 Toeplitz matmuls on d = s_cur - s_prev
  DVE relu with folded alpha-weights: y4w = max(psum,0) * w_all -> y4 tiles
  S3  temporal integration: frame-chunk stationaries accumulate
      PF[frame-in-chunk, (Fchunk, pair)] on PE; PE-transpose to
      [pair, frame]; frame-rate scan (alpha^256) on DVE; DMA out.
"""

import numpy as np

NCH, BS, T = 129, 8, 64000
L = 128                      # time block
NBLK = T // L                # 500 blocks
NFRM = 250                   # output frames (stride 256)
NCORE = 8
CPC = 16                     # output channels per core
NPAIR = 128                  # (channel, batch) pairs per core
BETA = float(np.exp(-1.0 / 8.0))
ALPHA = float(np.exp(-1.0 / 128.0))
A256 = float(ALPHA ** 256)
KMAX = 1024
TAIL_TOL = 2e-3

_cache = {}


def _impulse_responses(coch_B, coch_A):
    """h[c, k] for k < KMAX, float64, from the order-4 IIR coefficients."""
    B = coch_B.astype(np.float64)
    A = coch_A.astype(np.float64)
    h = np.zeros((NCH, KMAX))
    for t in range(KMAX):
        acc = B[:, t].copy() if t < 5 else np.zeros(NCH)
        for k in range(1, 5):
            if t - k >= 0:
                acc -= A[:, k] * h[:, t - k]
        h[:, t] = acc
    return h


def _band_matrix(hc, b):
    """T_b[p_in, p_out] = h[128*b + p_out - p_in] (0 where the tap index < 0)."""
    p = np.arange(L)
    idx = 128 * b + p[None, :] - p[:, None]
    valid = idx >= 0
    out = np.where(valid, hc[np.clip(idx, 0, KMAX - 1)], 0.0)
    return out


def _host_prep(wavData, coch_B, coch_A):
    wavData = np.asarray(wavData, dtype=np.float32)
    coch_B = np.asarray(coch_B, dtype=np.float64)
    coch_A = np.asarray(coch_A, dtype=np.float64)
    h = _impulse_responses(coch_B, coch_A)
    tails = np.cumsum(np.abs(h[:, ::-1]), axis=1)[:, ::-1]
    taps = np.array([
        int(np.argmax(tails[c] < TAIL_TOL)) if tails[c, 0] >= TAIL_TOL else 1
        for c in range(NCH)
    ])
    nb = np.clip(np.ceil(taps / 128.0).astype(int), 1, 8)
    # SPMD: every core runs the same program, so band counts must be uniform
    # per local channel position (max across cores).
    nb_u = [max(int(nb[CPC * k + i]) for k in range(NCORE)) for i in range(CPC + 1)]
    nbtot = sum(nb_u)
    woff = np.cumsum([0] + nb_u)

    # x: [128 pos, (bs, block)] fp16, same for all cores
    x16 = np.ascontiguousarray(
        wavData.reshape(BS, NBLK, L).transpose(2, 0, 1).reshape(L, BS * NBLK)
    ).astype(np.float16)

    w1s = []
    for k in range(NCORE):
        W1 = np.zeros((L, nbtot * L), np.float16)
        for i in range(CPC + 1):
            c = CPC * k + i
            for b in range(nb_u[i]):
                W1[:, (woff[i] + b) * L:(woff[i] + b + 1) * L] = \
                    _band_matrix(h[c], b).astype(np.float16)
        w1s.append(W1)

    p = np.arange(L)
    T0 = np.where(p[None, :] >= p[:, None], BETA ** (p[None, :] - p[:, None]), 0.0)
    T1 = np.where(p[:, None] > p[None, :], BETA ** (128 + p[None, :] - p[:, None]), 0.0)
    WB = np.concatenate([T0, T1], axis=1).astype(np.float16)

    # S3 stationaries. Matmul outputs must start at partition 0, so the
    # target frame-row m is selected by leading zero columns: slice
    # W3E[:, 16-m : 18] gives [zeros*m | e0 | ones(p>=1)] (width m+2) and
    # W3O[:, 16-m : 17] gives [zeros*m | ones] (width m+1).
    W3E = np.zeros((L, 18), np.float16)
    W3E[0, 16] = 1.0          # e0: the frame sample itself
    W3E[1:, 17] = 1.0         # prev-prev block (alpha folded into y4)
    W3O = np.zeros((L, 17), np.float16)
    W3O[:, 16] = 1.0          # prev block

    # relu weight fold: even blocks get the prev-prev profile alpha^(256-p)
    # except p=0 which serves e0 (weight 1); odd blocks get alpha^(128-p).
    WALL = np.zeros((L, NBLK), np.float32)
    WALL[:, 0::2] = (ALPHA ** (256.0 - p))[:, None]
    WALL[0, 0::2] = 1.0
    WALL[:, 1::2] = (ALPHA ** (128.0 - p))[:, None]
    WALL = WALL.astype(np.float16)

    EYE16 = np.eye(16, dtype=np.float16)

    ins = [dict(x=x16, w1=w1s[k], wb=WB, w3e=W3E, w3o=W3O, wall=WALL,
                eye16=EYE16)
           for k in range(NCORE)]
    return ins, tuple(nb_u)


def _build(nb_u, dyn_rep=1):
    import contextlib
    import concourse.bacc as bacc
    import concourse.tile as tile
    from concourse import mybir
    from concourse.ap import AP

    nbtot = sum(nb_u)
    woff = np.cumsum([0] + list(nb_u))
    f16, f32 = mybir.dt.float16, mybir.dt.float32

    nc = bacc.Bacc("TRN2", target_bir_lowering=False, debug=False,
                   num_devices=NCORE)
    x_d = nc.dram_tensor("x", [L, BS * NBLK], f16, kind="ExternalInput")
    w1_d = nc.dram_tensor("w1", [L, nbtot * L], f16, kind="ExternalInput")
    wb_d = nc.dram_tensor("wb", [L, 256], f16, kind="ExternalInput")
    w3e_d = nc.dram_tensor("w3e", [L, 18], f16, kind="ExternalInput")
    w3o_d = nc.dram_tensor("w3o", [L, 17], f16, kind="ExternalInput")
    wall_d = nc.dram_tensor("wall", [L, NBLK], f16, kind="ExternalInput")
    eye_d = nc.dram_tensor("eye16", [16, 16], f16, kind="ExternalInput")
    out_d = nc.dram_tensor("out", [NPAIR, NFRM], f32, kind="ExternalOutput")

    def ap3(base_ap, off, dims):
        return AP(tensor=base_ap.tensor, offset=off,
                  ap=[list(base_ap.ap[0])] + [list(d) for d in dims])

    with tile.TileContext(nc) as tc:
        with tc.tile_pool(name="const", bufs=1) as cp, \
             tc.tile_pool(name="dp", bufs=2) as dp, \
             tc.tile_pool(name="ps1", bufs=1, space="PSUM") as ps1p, \
             tc.tile_pool(name="ps2", bufs=1, space="PSUM") as ps2p, \
             tc.tile_pool(name="ps3", bufs=1, space="PSUM") as ps3p, \
             tc.tile_pool(name="ps4", bufs=1, space="PSUM") as ps4p:
            x_sb = cp.tile([L, BS * NBLK], f16, name="x_sb")
            w1_sb = cp.tile([L, nbtot * L], f16, name="w1_sb")
            wb_sb = cp.tile([L, 256], f16, name="wb_sb")
            w3e_sb = cp.tile([L, 18], f16, name="w3e_sb")
            w3o_sb = cp.tile([L, 17], f16, name="w3o_sb")
            wall_sb = cp.tile([L, NBLK], f16, name="wall_sb")
            eye_sb = cp.tile([16, 16], f16, name="eye_sb")
            zr16 = cp.tile([1, 16], f16, name="zr16")
            s_a = cp.tile([L, BS * NBLK], f16, name="s_a")
            s_b = cp.tile([L, BS * NBLK], f16, name="s_b")
            y4a = cp.tile([L, 64 * NBLK], f16, name="y4a")
            y4b = cp.tile([L, 64 * NBLK], f16, name="y4b")
            pfs = cp.tile([16, 512], f16, name="pfs")
            gst = cp.tile([NPAIR, 256], f32, name="gst")
            acst = cp.tile([NPAIR, 256], f32, name="acst")
            f_sb = cp.tile([NPAIR, 256], f32, name="f_sb")

            nc.sync.dma_start(x_sb[:], x_d.ap())
            nc.sync.dma_start(w1_sb[:], w1_d.ap())
            nc.sync.dma_start(wb_sb[:], wb_d.ap())
            nc.sync.dma_start(w3e_sb[:], w3e_d.ap())
            nc.sync.dma_start(w3o_sb[:], w3o_d.ap())
            nc.sync.dma_start(wall_sb[:], wall_d.ap())
            nc.sync.dma_start(eye_sb[:], eye_d.ap())
            nc.vector.memset(zr16[:], 0.0)
            nc.vector.memset(acst[:], A256)

            loop_ctx = (tc.For_i(0, dyn_rep, 1) if dyn_rep > 1
                        else contextlib.nullcontext())
            with loop_ctx:
              for rep in range(1):

                def s1_round(ci, rnd):
                    """S1 matmuls for channel ci, batches 4*rnd..4*rnd+3,
                    followed by one fused sigmoid into s tile."""
                    s_cur = (s_a, s_b)[ci % 2]
                    nb = nb_u[ci]
                    ps = ps1p.tile([L, 2048], f32, name=f"ps1_{ci}_{rnd}",
                                   tag="s1")
                    for b in range(nb):
                        wap = w1_sb[:, (woff[ci] + b) * L:
                                    (woff[ci] + b + 1) * L]
                        for i in range(4):
                            bs = 4 * rnd + i
                            nc.tensor.matmul(
                                ps[:, i * 512 + b: i * 512 + NBLK], wap,
                                x_sb[:, bs * NBLK: bs * NBLK + NBLK - b],
                                start=(b == 0), stop=(b == nb - 1))
                    src = ap3(ps[:], 0, [[512, 4], [1, NBLK]])
                    nc.scalar.activation(
                        s_cur[:, rnd * 2000: rnd * 2000 + 2000], src,
                        mybir.ActivationFunctionType.Sigmoid)

                def s2_flight(p, fl, d4s):
                    """S2 for pairs (channel p, bs=2*fl) and (p, bs=2*fl+1)."""
                    y4h = y4a if p <= 8 else y4b
                    ps = ps2p.tile([L, 1024], f32, name=f"ps2_{p}_{fl}",
                                   tag="s2")
                    d4 = d4s[fl // 2]
                    for k in range(2):
                        bs = 2 * fl + k
                        dsl = d4[:, (bs % 4) * NBLK: (bs % 4) * NBLK + NBLK]
                        nc.tensor.matmul(ps[:, k * 512: k * 512 + NBLK],
                                         wb_sb[:, 0:L], dsl,
                                         start=True, stop=False)
                    for k in range(2):
                        bs = 2 * fl + k
                        dsl = d4[:, (bs % 4) * NBLK: (bs % 4) * NBLK + NBLK - 1]
                        nc.tensor.matmul(ps[:, k * 512 + 1: k * 512 + NBLK],
                                         wb_sb[:, L:2 * L], dsl,
                                         start=False, stop=True)
                    for k in range(2):
                        bs = 2 * fl + k
                        pl = ((p - 1) * 8 + bs) % 64
                        nc.vector.scalar_tensor_tensor(
                            y4h[:, pl * NBLK: pl * NBLK + NBLK],
                            ps[:, k * 512: k * 512 + NBLK], 0.0,
                            wall_sb[:], mybir.AluOpType.max,
                            mybir.AluOpType.mult)

                def s3_half(half):
                    y4h = (y4a, y4b)[half]
                    y4ap = y4h[:]
                    for grp in range(2):
                        pf = ps3p.tile([16, 512], f32,
                                       name=f"pf_{half}_{grp}", tag="pf")
                        # zero-init the whole PF region
                        nc.tensor.matmul(pf[0:16, 0:512], zr16[0:1, 0:16],
                                         x_sb[0:1, 0:512],
                                         start=True, stop=False)
                        # even r: stationary [e0 | ones(p>=1)] -> rows
                        # r/2, r/2+1; odd r: [ones] -> row (r+1)/2
                        mms = []
                        for r in range(0, 31, 2):     # even, ascending
                            m = r // 2
                            if m + 1 > 15:
                                mms.append((r, w3e_sb[:, 16 - m:17], m))
                            else:
                                mms.append((r, w3e_sb[:, 16 - m:18], m + 1))
                        for r in range(1, 31, 2):     # odd
                            m = (r + 1) // 2
                            mms.append((r, w3o_sb[:, 16 - m:17], m))
                        mms.append((-2, w3e_sb[:, 17:18], 0))
                        mms.append((-1, w3o_sb[:, 16:17], 0))
                        n_mm = len(mms)
                        for idx, (r, wap, mtop) in enumerate(mms):
                            if grp == 0 and r < 0:
                                src = ap3(y4ap, 32 + r,
                                          [[32, 7], [NBLK, 64]])
                                dst = pf[0:mtop + 1, 64:512]
                            elif grp == 1 and r >= 19:
                                src = ap3(y4ap, 256 + r,
                                          [[32, 7], [NBLK, 64]])
                                dst = pf[0:mtop + 1, 0:448]
                            else:
                                src = ap3(y4ap, 256 * grp + r,
                                          [[32, 8], [NBLK, 64]])
                                dst = pf[0:mtop + 1, 0:512]
                            nc.tensor.matmul(dst, wap, src, start=False,
                                             stop=(idx == n_mm - 1))
                        # PF -> sbuf fp16
                        nc.vector.tensor_scalar_max(pfs[:], pf[:], 0.0)
                        # transpose each [16, 64] pair-block -> [64, 16]
                        pst = ps4p.tile([NPAIR, 128], f16,
                                        name=f"pst_{half}_{grp}", tag="pst")
                        base = half * 64
                        for fi in range(8):
                            if grp == 1 and fi == 7:
                                # frames 240..249 only live in cols 0..9
                                pass
                            nc.tensor.transpose(
                                pst[base:base + 64, fi * 16: fi * 16 + 16],
                                pfs[0:16, fi * 64: fi * 64 + 64],
                                eye_sb[:])
                        ncol = 128 if grp == 0 else 122
                        nc.vector.tensor_scalar_add(
                            gst[base:base + 64,
                                128 * grp: 128 * grp + ncol],
                            pst[base:base + 64, 0:ncol], 0.0)

                # iteration ci: S1+sigmoid for channel ci; S2 for pair p=ci-1
                # (whose s tiles were finished in earlier iterations). The S2
                # matmuls sit between S1 rounds so the PE never waits for the
                # Act engine to drain ps1.
                for ci in range(CPC + 2):
                    p = ci - 1
                    d4s = []
                    if p >= 1:
                        s_cur = (s_a, s_b)[p % 2]
                        s_prev = (s_a, s_b)[(p + 1) % 2]
                        for rnd in range(2):
                            d4 = dp.tile([L, 2000], f16,
                                         name=f"d4_{p}_{rnd}", tag=f"d{rnd}")
                            nc.vector.tensor_sub(
                                d4[:],
                                s_cur[:, rnd * 2000: rnd * 2000 + 2000],
                                s_prev[:, rnd * 2000: rnd * 2000 + 2000])
                            d4s.append(d4)
                    if ci == 10:
                        s3_half(0)
                    if ci <= CPC:
                        s1_round(ci, 0)
                    if p >= 1:
                        s2_flight(p, 0, d4s)
                        s2_flight(p, 1, d4s)
                    if ci <= CPC:
                        s1_round(ci, 1)
                    if p >= 1:
                        s2_flight(p, 2, d4s)
                        s2_flight(p, 3, d4s)
                s3_half(1)
                nc.vector.tensor_tensor_scan(
                    f_sb[:, 0:NFRM], acst[:, 0:NFRM], gst[:, 0:NFRM],
                    0.0, mybir.AluOpType.mult, mybir.AluOpType.add)
                nc.sync.dma_start(out_d.ap(), f_sb[:, 0:NFRM])
    _dedupe_ldweights(nc)
    nc.compile()
    return nc


def _dedupe_ldweights(nc):
    """Drop PE weight loads whose stationary operand matches the previous
    load in the scheduled PE stream (the splitter emits one per matmul)."""
    from concourse import mybir
    dropped = 0
    for bb in nc.m.functions[0].blocks:
        last_key = None
        keep = []
        for inst in bb.instructions:
            if isinstance(inst, mybir.InstLdweights):
                si = inst.sync_info
                key = str(inst.ins[0])
                if (key == last_key and not (si and (si.on_wait or si.on_update))):
                    dropped += 1
                    continue
                last_key = key
            elif isinstance(inst, (mybir.InstUnconditionalBranch,
                                   mybir.InstCompareAndBranch)):
                last_key = None
            keep.append(inst)
        if len(keep) != len(bb.instructions):
            bb.instructions = keep
    return dropped


def _make_runner(nc):
    """Persistent jitted 8-core runner (mirrors bass2jax.run_bass_via_pjrt)."""
    import jax
    from jax.sharding import Mesh, PartitionSpec
    from jax.experimental.shard_map import shard_map
    from concourse import bass2jax, mybir

    bass2jax.install_neuronx_cc_hook()

    partition_name = (
        nc.partition_id_tensor.name if nc.partition_id_tensor else None
    )
    in_names, out_names, out_avals, zero_shapes = [], [], [], []
    for alloc in nc.m.functions[0].allocations:
        if not isinstance(alloc, mybir.MemoryLocationSet):
            continue
        name = alloc.memorylocations[0].name
        if alloc.kind == "ExternalInput":
            if name != partition_name:
                in_names.append(name)
        elif alloc.kind == "ExternalOutput":
            out_names.append(name)
            shape = tuple(alloc.tensor_shape)
            dtype = mybir.dt.np(alloc.dtype)
            out_avals.append(jax.core.ShapedArray(shape, dtype))
            zero_shapes.append((shape, dtype))
    n_params = len(in_names)
    all_in_names = list(in_names) + list(out_names)
    if partition_name is not None:
        all_in_names.append(partition_name)

    def _body(*args):
        operands = list(args)
        if partition_name is not None:
            operands.append(bass2jax.partition_id_tensor())
        outs = bass2jax._bass_exec_p.bind(
            *operands,
            out_avals=tuple(out_avals),
            in_names=tuple(all_in_names),
            out_names=tuple(out_names),
            lowering_input_output_aliases=(),
            sim_require_finite=True,
            sim_require_nnan=True,
            nc=nc,
        )
        return tuple(outs)

    devices = jax.devices()[:NCORE]
    mesh = Mesh(np.asarray(devices), ("core",))
    n_outs = len(out_names)
    sharded = jax.jit(
        shard_map(_body, mesh=mesh,
                  in_specs=(PartitionSpec("core"),) * (n_params + n_outs),
                  out_specs=(PartitionSpec("core"),) * n_outs,
                  check_rep=False),
        donate_argnums=tuple(range(n_params, n_params + n_outs)),
        keep_unused=True,
    )

    def run(in_maps):
        concat_in = [
            np.concatenate([np.asarray(m[name]) for m in in_maps], axis=0)
            for name in in_names
        ]
        concat_zeros = [
            np.zeros((NCORE * s[0], *s[1:]), d) for (s, d) in zero_shapes
        ]
        out_arrs = sharded(*concat_in, *concat_zeros)
        return [
            {name: np.asarray(out_arrs[i]).reshape(NCORE, *out_avals[i].shape)[c]
             for i, name in enumerate(out_names)}
            for c in range(NCORE)
        ]

    return run


def _get_runner(wavData, coch_B, coch_A):
    in_maps, nb_u = _host_prep(wavData, coch_B, coch_A)
    if nb_u not in _cache:
        nc = _build(nb_u)
        _cache[nb_u] = _make_runner(nc)
    return _cache[nb_u], in_maps


def kernel(wavData, coch_B, coch_A):
    run, in_maps = _get_runner(wavData, coch_B, coch_A)
    results = run(in_maps)
    out = np.zeros((BS, NCH, NFRM), np.float32)
    for k in range(NCORE):
        F = results[k]["out"]                      # [128 pairs, 250]
        out[:, CPC * k + 1: CPC * (k + 1) + 1, :] = \
            F.reshape(CPC, BS, NFRM).transpose(1, 0, 2)
    return out


# revision 13
# speedup vs baseline: 1.1449x; 1.1449x over previous
"""Auditory spectrogram kernel for Trainium2 (8 NeuronCores, Bass/Tile).

Pipeline per the reference:
  y1 = order-4 IIR cochlear filterbank (129 channels, per-channel B/A) over wav [8, 64000]
  y2 = sigmoid(y1); y2 = 1st-order IIR (beta) over time
  y4 = relu(y2[c] - y2[c-1]); y5 = 1st-order IIR (alpha); downsample every 256 -> [8, 129, 250]

All linear recurrences are blocked-FIR matmuls on TensorE (fp16 operands,
fp32 PSUM). Channel 0's output is exactly zero, so the 128 real output
channels are sharded 16 per core; each core computes 16 channels + a
1-channel halo.

Layout: time blocked into 500 blocks of 128; partition = position in block,
free = (batch, block). Per core:
  S1  per-channel banded-Toeplitz matmuls -> psum [128, 4x512] (4 batches)
  Act one fused sigmoid per 4 batches -> s tiles fp16
  S2  hair-cell LPF: T0/T1 Toeplitz matmuls on d = s_cur - s_prev
  DVE relu with folded alpha-weights: y4w = max(psum,0) * w_all -> y4 tiles
  S3  temporal integration: frame-chunk stationaries accumulate
      PF[frame-in-chunk, (Fchunk, pair)] on PE; PE-transpose to
      [pair, frame]; frame-rate scan (alpha^256) on DVE; DMA out.
"""

import numpy as np

NCH, BS, T = 129, 8, 64000
L = 128                      # time block
NBLK = T // L                # 500 blocks
NFRM = 250                   # output frames (stride 256)
NCORE = 8
CPC = 16                     # output channels per core
NPAIR = 128                  # (channel, batch) pairs per core
BETA = float(np.exp(-1.0 / 8.0))
ALPHA = float(np.exp(-1.0 / 128.0))
A256 = float(ALPHA ** 256)
KMAX = 1024
TAIL_TOL = 2e-3

_cache = {}


def _impulse_responses(coch_B, coch_A):
    """h[c, k] for k < KMAX, float64, from the order-4 IIR coefficients."""
    B = coch_B.astype(np.float64)
    A = coch_A.astype(np.float64)
    h = np.zeros((NCH, KMAX))
    for t in range(KMAX):
        acc = B[:, t].copy() if t < 5 else np.zeros(NCH)
        for k in range(1, 5):
            if t - k >= 0:
                acc -= A[:, k] * h[:, t - k]
        h[:, t] = acc
    return h


def _band_matrix(hc, b):
    """T_b[p_in, p_out] = h[128*b + p_out - p_in] (0 where the tap index < 0)."""
    p = np.arange(L)
    idx = 128 * b + p[None, :] - p[:, None]
    valid = idx >= 0
    out = np.where(valid, hc[np.clip(idx, 0, KMAX - 1)], 0.0)
    return out


def _host_prep(wavData, coch_B, coch_A):
    wavData = np.asarray(wavData, dtype=np.float32)
    coch_B = np.asarray(coch_B, dtype=np.float64)
    coch_A = np.asarray(coch_A, dtype=np.float64)
    h = _impulse_responses(coch_B, coch_A)
    tails = np.cumsum(np.abs(h[:, ::-1]), axis=1)[:, ::-1]
    taps = np.array([
        int(np.argmax(tails[c] < TAIL_TOL)) if tails[c, 0] >= TAIL_TOL else 1
        for c in range(NCH)
    ])
    nb = np.clip(np.ceil(taps / 128.0).astype(int), 1, 8)
    # SPMD: every core runs the same program, so band counts must be uniform
    # per local channel position (max across cores).
    nb_u = [max(int(nb[CPC * k + i]) for k in range(NCORE)) for i in range(CPC + 1)]
    nbtot = sum(nb_u)
    woff = np.cumsum([0] + nb_u)

    # x: [128 pos, (bs, block)] fp16, same for all cores
    x16 = np.ascontiguousarray(
        wavData.reshape(BS, NBLK, L).transpose(2, 0, 1).reshape(L, BS * NBLK)
    ).astype(np.float16)

    w1s = []
    for k in range(NCORE):
        W1 = np.zeros((L, nbtot * L), np.float16)
        for i in range(CPC + 1):
            c = CPC * k + i
            for b in range(nb_u[i]):
                W1[:, (woff[i] + b) * L:(woff[i] + b + 1) * L] = \
                    _band_matrix(h[c], b).astype(np.float16)
        w1s.append(W1)

    p = np.arange(L)
    T0 = np.where(p[None, :] >= p[:, None], BETA ** (p[None, :] - p[:, None]), 0.0)
    T1 = np.where(p[:, None] > p[None, :], BETA ** (128 + p[None, :] - p[:, None]), 0.0)
    WB = np.concatenate([T0, T1], axis=1).astype(np.float16)

    # S3 stationaries. Matmul outputs must start at partition 0, so the
    # target row m inside the 16-pair psum group is selected by leading
    # zero columns: slice W[:, 16-m : 17] = [zeros*m | profile].
    # (alpha weights are folded into y4 by the relu, so profiles are 0/1.)
    W3E0 = np.zeros((L, 33), np.float16)
    W3E0[0, 32] = 1.0         # e0: the frame sample itself
    W3PP = np.zeros((L, 33), np.float16)
    W3PP[1:, 32] = 1.0        # prev-prev block, p>=1
    W3PR = np.zeros((L, 33), np.float16)
    W3PR[:, 32] = 1.0         # prev block

    # relu weight fold: even blocks get the prev-prev profile alpha^(256-p)
    # except p=0 which serves e0 (weight 1); odd blocks get alpha^(128-p).
    WALL = np.zeros((L, NBLK), np.float32)
    WALL[:, 0::2] = (ALPHA ** (256.0 - p))[:, None]
    WALL[0, 0::2] = 1.0
    WALL[:, 1::2] = (ALPHA ** (128.0 - p))[:, None]
    WALL = WALL.astype(np.float16)

    ins = [dict(x=x16, w1=w1s[k], wb=WB, w3e0=W3E0, w3pp=W3PP, w3pr=W3PR,
                wall=WALL)
           for k in range(NCORE)]
    return ins, tuple(nb_u)


def _build(nb_u, dyn_rep=1, stage='full'):
    import contextlib
    import concourse.bacc as bacc
    import concourse.tile as tile
    from concourse import mybir
    from concourse.ap import AP

    nbtot = sum(nb_u)
    woff = np.cumsum([0] + list(nb_u))
    f16, f32 = mybir.dt.float16, mybir.dt.float32

    nc = bacc.Bacc("TRN2", target_bir_lowering=False, debug=False,
                   num_devices=NCORE)
    x_d = nc.dram_tensor("x", [L, BS * NBLK], f16, kind="ExternalInput")
    w1_d = nc.dram_tensor("w1", [L, nbtot * L], f16, kind="ExternalInput")
    wb_d = nc.dram_tensor("wb", [L, 256], f16, kind="ExternalInput")
    w3e0_d = nc.dram_tensor("w3e0", [L, 33], f16, kind="ExternalInput")
    w3pp_d = nc.dram_tensor("w3pp", [L, 33], f16, kind="ExternalInput")
    w3pr_d = nc.dram_tensor("w3pr", [L, 33], f16, kind="ExternalInput")
    wall_d = nc.dram_tensor("wall", [L, NBLK], f16, kind="ExternalInput")
    out_d = nc.dram_tensor("out", [NPAIR, NFRM], f32, kind="ExternalOutput")

    def ap3(base_ap, off, dims):
        return AP(tensor=base_ap.tensor, offset=off,
                  ap=[list(base_ap.ap[0])] + [list(d) for d in dims])

    with tile.TileContext(nc) as tc:
        with tc.tile_pool(name="const", bufs=1) as cp, \
             tc.tile_pool(name="dp", bufs=2) as dp, \
             tc.tile_pool(name="wp", bufs=4) as wp, \
             tc.tile_pool(name="ps1", bufs=2, space="PSUM") as ps1p, \
             tc.tile_pool(name="ps2", bufs=1, space="PSUM") as ps2p, \
             tc.tile_pool(name="ps3", bufs=2, space="PSUM") as ps3p:
            x_sb = cp.tile([L, BS * NBLK], f16, name="x_sb")
            w1_sb = cp.tile([L, nbtot * L], f16, name="w1_sb")
            wb_sb = cp.tile([L, 256], f16, name="wb_sb")
            w3e0_sb = cp.tile([L, 33], f16, name="w3e0_sb")
            w3pp_sb = cp.tile([L, 33], f16, name="w3pp_sb")
            w3pr_sb = cp.tile([L, 33], f16, name="w3pr_sb")
            wall_sb = cp.tile([L, NBLK], f16, name="wall_sb")
            zr32 = cp.tile([1, 32], f16, name="zr32")
            s_a = cp.tile([L, BS * NBLK], f16, name="s_a")
            s_b = cp.tile([L, BS * NBLK], f16, name="s_b")
            gst = cp.tile([NPAIR, 256], f32, name="gst")
            acst = cp.tile([NPAIR, 256], f32, name="acst")
            f_sb = cp.tile([NPAIR, 256], f32, name="f_sb")

            nc.sync.dma_start(x_sb[:], x_d.ap())
            nc.sync.dma_start(w1_sb[:], w1_d.ap())
            nc.sync.dma_start(wb_sb[:], wb_d.ap())
            nc.sync.dma_start(w3e0_sb[:], w3e0_d.ap())
            nc.sync.dma_start(w3pp_sb[:], w3pp_d.ap())
            nc.sync.dma_start(w3pr_sb[:], w3pr_d.ap())
            nc.sync.dma_start(wall_sb[:], wall_d.ap())
            nc.vector.memset(zr32[:], 0.0)
            nc.vector.memset(acst[:], A256)

            loop_ctx = (tc.For_i(0, dyn_rep, 1) if dyn_rep > 1
                        else contextlib.nullcontext())
            with loop_ctx:
              for rep in range(1):

                def s1_round(ci, rnd):
                    """S1 matmuls for channel ci, batches 2*rnd..2*rnd+1,
                    followed by one fused sigmoid into s tile."""
                    s_cur = (s_a, s_b)[ci % 2]
                    nb = nb_u[ci]
                    ps = ps1p.tile([L, 1024], f32, name=f"ps1_{ci}_{rnd}",
                                   tag="s1")
                    for b in range(nb):
                        wap = w1_sb[:, (woff[ci] + b) * L:
                                    (woff[ci] + b + 1) * L]
                        for i in range(2):
                            bs = 2 * rnd + i
                            nc.tensor.matmul(
                                ps[:, i * 512 + b: i * 512 + NBLK], wap,
                                x_sb[:, bs * NBLK: bs * NBLK + NBLK - b],
                                start=(b == 0), stop=(b == nb - 1))
                    src = ap3(ps[:], 0, [[512, 2], [1, NBLK]])
                    nc.scalar.activation(
                        s_cur[:, rnd * 1000: rnd * 1000 + 1000], src,
                        mybir.ActivationFunctionType.Sigmoid)

                pf_box = [None]

                def s2_flight(p, fl, d4s):
                    """S2 + relu + S3 for pairs (p, bs=2*fl), (p, bs=2*fl+1).

                    S3: each pair contributes one frame row to the current
                    16-pair psum group pf16 [16, 250]; the row is selected
                    by leading zero columns in the stationaries.
                    """
                    ps = ps2p.tile([L, 1024], f32, name=f"ps2_{p}_{fl}",
                                   tag="s2")
                    d4 = d4s[fl // 2]
                    for k in range(2):
                        bs = 2 * fl + k
                        dsl = d4[:, (bs % 4) * NBLK: (bs % 4) * NBLK + NBLK]
                        nc.tensor.matmul(ps[:, k * 512: k * 512 + NBLK],
                                         wb_sb[:, 0:L], dsl,
                                         start=True, stop=False)
                    for k in range(2):
                        bs = 2 * fl + k
                        dsl = d4[:, (bs % 4) * NBLK: (bs % 4) * NBLK + NBLK - 1]
                        nc.tensor.matmul(ps[:, k * 512 + 1: k * 512 + NBLK],
                                         wb_sb[:, L:2 * L], dsl,
                                         start=False, stop=True)
                    y4t = []
                    for k in range(2):
                        bs = 2 * fl + k
                        y4 = wp.tile([L, NBLK], f16,
                                     name=f"y4_{p}_{bs}", tag=f"y4{bs % 4}")
                        nc.vector.scalar_tensor_tensor(
                            y4[:], ps[:, k * 512: k * 512 + NBLK], 0.0,
                            wall_sb[:], mybir.AluOpType.max,
                            mybir.AluOpType.mult)
                        y4t.append(y4)
                    if stage != 'full':
                        for k in range(2):
                            nc.vector.tensor_scalar_add(
                                gst[0:128, fl * 8 + k * 4: fl * 8 + k * 4 + 4],
                                y4t[k][0:128, 0:4], 0.0)
                        return
                    for k in range(2):
                        bs = 2 * fl + k
                        row = (p - 1) * 8 + bs
                        g, m = row // 32, row % 32
                        if m == 0:
                            pf_box[0] = ps3p.tile([32, 256], f32,
                                                  name=f"pf32_{g}", tag="pf")
                            nc.tensor.matmul(pf_box[0][0:32, 0:NFRM],
                                             zr32[0:1, 0:32],
                                             x_sb[0:1, 0:NFRM],
                                             start=True, stop=False)
                        pf = pf_box[0]
                        y4 = y4t[k]
                        last = (m == 31)
                        nc.tensor.matmul(pf[0:m + 1, 0:NFRM],
                                         w3e0_sb[:, 32 - m:33],
                                         y4[:, 0:2 * NFRM:2],
                                         start=False, stop=False)
                        nc.tensor.matmul(pf[0:m + 1, 1:NFRM],
                                         w3pp_sb[:, 32 - m:33],
                                         y4[:, 0:2 * NFRM - 2:2],
                                         start=False, stop=False)
                        nc.tensor.matmul(pf[0:m + 1, 1:NFRM],
                                         w3pr_sb[:, 32 - m:33],
                                         y4[:, 1:2 * NFRM - 1:2],
                                         start=False, stop=last)
                        if last:
                            nc.vector.tensor_scalar_add(
                                gst[32 * g: 32 * g + 32, 0:NFRM],
                                pf[0:32, 0:NFRM], 0.0)

                # iteration ci: S1+sigmoid for channel ci; S2+S3 for pair
                # p=ci-1 (whose s tiles were finished in earlier iterations).
                # The S2/S3 matmuls sit between S1 rounds so the PE never
                # waits for the Act engine to drain ps1.
                for ci in range(CPC + 2):
                    p = ci - 1
                    d4s = []
                    if p >= 1 and stage != 's1':
                        s_cur = (s_a, s_b)[p % 2]
                        s_prev = (s_a, s_b)[(p + 1) % 2]
                        for rnd in range(2):
                            d4 = dp.tile([L, 2000], f16,
                                         name=f"d4_{p}_{rnd}", tag=f"d{rnd}")
                            nc.vector.tensor_sub(
                                d4[:],
                                s_cur[:, rnd * 2000: rnd * 2000 + 2000],
                                s_prev[:, rnd * 2000: rnd * 2000 + 2000])
                            d4s.append(d4)
                    for rnd in range(4):
                        if ci <= CPC:
                            s1_round(ci, rnd)
                        if p >= 1 and stage not in ('s1',):
                            s2_flight(p, rnd, d4s)
                if stage != 'full':
                    # consume s tiles so nothing is dead-code eliminated
                    nc.vector.tensor_scalar_add(gst[0:128, 0:4], s_a[:, 0:4], 0.0)
                    nc.vector.tensor_scalar_add(gst[0:128, 4:8], s_b[:, 0:4], 0.0)
                if stage == 'full':
                    nc.vector.tensor_tensor_scan(
                        f_sb[:, 0:NFRM], acst[:, 0:NFRM], gst[:, 0:NFRM],
                        0.0, mybir.AluOpType.mult, mybir.AluOpType.add)
                    nc.sync.dma_start(out_d.ap(), f_sb[:, 0:NFRM])
                else:
                    nc.sync.dma_start(out_d.ap(), gst[:, 0:NFRM])
    _dedupe_ldweights(nc)
    nc.compile()
    return nc


def _dedupe_ldweights(nc):
    """Drop PE weight loads whose stationary operand matches the previous
    load in the scheduled PE stream (the splitter emits one per matmul)."""
    from concourse import mybir
    dropped = 0
    for bb in nc.m.functions[0].blocks:
        last_key = None
        keep = []
        for inst in bb.instructions:
            if isinstance(inst, mybir.InstLdweights):
                si = inst.sync_info
                key = str(inst.ins[0])
                if (key == last_key and not (si and (si.on_wait or si.on_update))):
                    dropped += 1
                    continue
                last_key = key
            elif isinstance(inst, (mybir.InstUnconditionalBranch,
                                   mybir.InstCompareAndBranch)):
                last_key = None
            keep.append(inst)
        if len(keep) != len(bb.instructions):
            bb.instructions = keep
    return dropped


def _make_runner(nc):
    """Persistent jitted 8-core runner (mirrors bass2jax.run_bass_via_pjrt)."""
    import jax
    from jax.sharding import Mesh, PartitionSpec
    from jax.experimental.shard_map import shard_map
    from concourse import bass2jax, mybir

    bass2jax.install_neuronx_cc_hook()

    partition_name = (
        nc.partition_id_tensor.name if nc.partition_id_tensor else None
    )
    in_names, out_names, out_avals, zero_shapes = [], [], [], []
    for alloc in nc.m.functions[0].allocations:
        if not isinstance(alloc, mybir.MemoryLocationSet):
            continue
        name = alloc.memorylocations[0].name
        if alloc.kind == "ExternalInput":
            if name != partition_name:
                in_names.append(name)
        elif alloc.kind == "ExternalOutput":
            out_names.append(name)
            shape = tuple(alloc.tensor_shape)
            dtype = mybir.dt.np(alloc.dtype)
            out_avals.append(jax.core.ShapedArray(shape, dtype))
            zero_shapes.append((shape, dtype))
    n_params = len(in_names)
    all_in_names = list(in_names) + list(out_names)
    if partition_name is not None:
        all_in_names.append(partition_name)

    def _body(*args):
        operands = list(args)
        if partition_name is not None:
            operands.append(bass2jax.partition_id_tensor())
        outs = bass2jax._bass_exec_p.bind(
            *operands,
            out_avals=tuple(out_avals),
            in_names=tuple(all_in_names),
            out_names=tuple(out_names),
            lowering_input_output_aliases=(),
            sim_require_finite=True,
            sim_require_nnan=True,
            nc=nc,
        )
        return tuple(outs)

    devices = jax.devices()[:NCORE]
    mesh = Mesh(np.asarray(devices), ("core",))
    n_outs = len(out_names)
    sharded = jax.jit(
        shard_map(_body, mesh=mesh,
                  in_specs=(PartitionSpec("core"),) * (n_params + n_outs),
                  out_specs=(PartitionSpec("core"),) * n_outs,
                  check_rep=False),
        donate_argnums=tuple(range(n_params, n_params + n_outs)),
        keep_unused=True,
    )

    def run(in_maps):
        concat_in = [
            np.concatenate([np.asarray(m[name]) for m in in_maps], axis=0)
            for name in in_names
        ]
        concat_zeros = [
            np.zeros((NCORE * s[0], *s[1:]), d) for (s, d) in zero_shapes
        ]
        out_arrs = sharded(*concat_in, *concat_zeros)
        return [
            {name: np.asarray(out_arrs[i]).reshape(NCORE, *out_avals[i].shape)[c]
             for i, name in enumerate(out_names)}
            for c in range(NCORE)
        ]

    return run


def _get_runner(wavData, coch_B, coch_A):
    in_maps, nb_u = _host_prep(wavData, coch_B, coch_A)
    if nb_u not in _cache:
        nc = _build(nb_u)
        _cache[nb_u] = _make_runner(nc)
    return _cache[nb_u], in_maps


def kernel(wavData, coch_B, coch_A):
    run, in_maps = _get_runner(wavData, coch_B, coch_A)
    results = run(in_maps)
    out = np.zeros((BS, NCH, NFRM), np.float32)
    for k in range(NCORE):
        F = results[k]["out"]                      # [128 pairs, 250]
        out[:, CPC * k + 1: CPC * (k + 1) + 1, :] = \
            F.reshape(CPC, BS, NFRM).transpose(1, 0, 2)
    return out


# revision 14
# speedup vs baseline: 1.4268x; 1.2462x over previous
"""Auditory spectrogram kernel for Trainium2 (8 NeuronCores, Bass/Tile).

Pipeline per the reference:
  y1 = order-4 IIR cochlear filterbank (129 channels, per-channel B/A) over wav [8, 64000]
  y2 = sigmoid(y1); y2 = 1st-order IIR (beta) over time
  y4 = relu(y2[c] - y2[c-1]); y5 = 1st-order IIR (alpha); downsample every 256 -> [8, 129, 250]

All linear recurrences are blocked-FIR matmuls on TensorE (fp16 operands,
fp32 PSUM). Channel 0's output is exactly zero, so the 128 real output
channels are sharded 16 per core; each core computes 16 channels + a
1-channel halo.

Layout: time blocked into 500 blocks of 128; partition = position in block,
free = (batch, block). Per core:
  S1  per-channel banded-Toeplitz matmuls -> psum [128, 4x512] (4 batches)
  Act one fused sigmoid per 4 batches -> s tiles fp16
  S2  hair-cell LPF: T0/T1 Toeplitz matmuls on d = s_cur - s_prev
  DVE relu with folded alpha-weights: y4w = max(psum,0) * w_all -> y4 tiles
  S3  temporal integration: frame-chunk stationaries accumulate
      PF[frame-in-chunk, (Fchunk, pair)] on PE; PE-transpose to
      [pair, frame]; frame-rate scan (alpha^256) on DVE; DMA out.
"""

import numpy as np

NCH, BS, T = 129, 8, 64000
L = 128                      # time block
NBLK = T // L                # 500 blocks
NFRM = 250                   # output frames (stride 256)
NCORE = 8
CPC = 16                     # output channels per core
NPAIR = 128                  # (channel, batch) pairs per core
BETA = float(np.exp(-1.0 / 8.0))
ALPHA = float(np.exp(-1.0 / 128.0))
A256 = float(ALPHA ** 256)
KMAX = 1024
TAIL_TOL = 2e-3

_cache = {}


def _impulse_responses(coch_B, coch_A):
    """h[c, k] for k < KMAX, float64, from the order-4 IIR coefficients."""
    B = coch_B.astype(np.float64)
    A = coch_A.astype(np.float64)
    h = np.zeros((NCH, KMAX))
    for t in range(KMAX):
        acc = B[:, t].copy() if t < 5 else np.zeros(NCH)
        for k in range(1, 5):
            if t - k >= 0:
                acc -= A[:, k] * h[:, t - k]
        h[:, t] = acc
    return h


def _band_matrix(hc, b):
    """T_b[p_in, p_out] = h[128*b + p_out - p_in] (0 where the tap index < 0)."""
    p = np.arange(L)
    idx = 128 * b + p[None, :] - p[:, None]
    valid = idx >= 0
    out = np.where(valid, hc[np.clip(idx, 0, KMAX - 1)], 0.0)
    return out


def _host_prep(wavData, coch_B, coch_A):
    wavData = np.asarray(wavData, dtype=np.float32)
    coch_B = np.asarray(coch_B, dtype=np.float64)
    coch_A = np.asarray(coch_A, dtype=np.float64)
    h = _impulse_responses(coch_B, coch_A)
    tails = np.cumsum(np.abs(h[:, ::-1]), axis=1)[:, ::-1]
    taps = np.array([
        int(np.argmax(tails[c] < TAIL_TOL)) if tails[c, 0] >= TAIL_TOL else 1
        for c in range(NCH)
    ])
    nb = np.clip(np.ceil(taps / 128.0).astype(int), 1, 8)
    # SPMD: every core runs the same program, so band counts must be uniform
    # per local channel position (max across cores).
    nb_u = [max(int(nb[CPC * k + i]) for k in range(NCORE)) for i in range(CPC + 1)]
    nbtot = sum(nb_u)
    woff = np.cumsum([0] + nb_u)

    # x: [128 pos, (bs, block)] fp16, same for all cores
    x16 = np.ascontiguousarray(
        wavData.reshape(BS, NBLK, L).transpose(2, 0, 1).reshape(L, BS * NBLK)
    ).astype(np.float16)

    w1s = []
    for k in range(NCORE):
        W1 = np.zeros((L, nbtot * L), np.float16)
        for i in range(CPC + 1):
            c = CPC * k + i
            for b in range(nb_u[i]):
                W1[:, (woff[i] + b) * L:(woff[i] + b + 1) * L] = \
                    _band_matrix(h[c], b).astype(np.float16)
        w1s.append(W1)

    p = np.arange(L)
    T0 = np.where(p[None, :] >= p[:, None], BETA ** (p[None, :] - p[:, None]), 0.0)
    T1 = np.where(p[:, None] > p[None, :], BETA ** (128 + p[None, :] - p[:, None]), 0.0)
    WB = np.concatenate([T0, T1], axis=1).astype(np.float16)

    # S3 stationaries. Matmul outputs must start at partition 0, so the
    # target row m inside the 16-pair psum group is selected by leading
    # zero columns: slice W[:, 16-m : 17] = [zeros*m | profile].
    # (alpha weights are folded into y4 by the relu, so profiles are 0/1.)
    W3E0 = np.zeros((L, 33), np.float16)
    W3E0[0, 32] = 1.0         # e0: the frame sample itself
    W3PP = np.zeros((L, 33), np.float16)
    W3PP[1:, 32] = 1.0        # prev-prev block, p>=1
    W3PR = np.zeros((L, 33), np.float16)
    W3PR[:, 32] = 1.0         # prev block

    # relu weight fold: even blocks get the prev-prev profile alpha^(256-p)
    # except p=0 which serves e0 (weight 1); odd blocks get alpha^(128-p).
    WALL = np.zeros((L, NBLK), np.float32)
    WALL[:, 0::2] = (ALPHA ** (256.0 - p))[:, None]
    WALL[0, 0::2] = 1.0
    WALL[:, 1::2] = (ALPHA ** (128.0 - p))[:, None]
    WALL = WALL.astype(np.float16)

    ins = [dict(x=x16, w1=w1s[k], wb=WB, w3e0=W3E0, w3pp=W3PP, w3pr=W3PR,
                wall=WALL)
           for k in range(NCORE)]
    return ins, tuple(nb_u)


def _build(nb_u, dyn_rep=1, stage='full'):
    import contextlib
    import concourse.bacc as bacc
    import concourse.tile as tile
    from concourse import mybir
    from concourse.ap import AP

    nbtot = sum(nb_u)
    woff = np.cumsum([0] + list(nb_u))
    f16, f32 = mybir.dt.float16, mybir.dt.float32

    nc = bacc.Bacc("TRN2", target_bir_lowering=False, debug=False,
                   num_devices=NCORE)
    x_d = nc.dram_tensor("x", [L, BS * NBLK], f16, kind="ExternalInput")
    w1_d = nc.dram_tensor("w1", [L, nbtot * L], f16, kind="ExternalInput")
    wb_d = nc.dram_tensor("wb", [L, 256], f16, kind="ExternalInput")
    w3e0_d = nc.dram_tensor("w3e0", [L, 33], f16, kind="ExternalInput")
    w3pp_d = nc.dram_tensor("w3pp", [L, 33], f16, kind="ExternalInput")
    w3pr_d = nc.dram_tensor("w3pr", [L, 33], f16, kind="ExternalInput")
    wall_d = nc.dram_tensor("wall", [L, NBLK], f16, kind="ExternalInput")
    out_d = nc.dram_tensor("out", [NPAIR, NFRM], f32, kind="ExternalOutput")

    def ap3(base_ap, off, dims):
        return AP(tensor=base_ap.tensor, offset=off,
                  ap=[list(base_ap.ap[0])] + [list(d) for d in dims])

    with tile.TileContext(nc) as tc:
        with tc.tile_pool(name="const", bufs=1) as cp, \
             tc.tile_pool(name="dp", bufs=2) as dp, \
             tc.tile_pool(name="wp", bufs=4) as wp, \
             tc.tile_pool(name="ps1", bufs=1, space="PSUM") as ps1p, \
             tc.tile_pool(name="ps2", bufs=3, space="PSUM") as ps2p, \
             tc.tile_pool(name="ps3", bufs=1, space="PSUM") as ps3p:
            x_sb = cp.tile([L, BS * NBLK], f16, name="x_sb")
            w1_sb = cp.tile([L, nbtot * L], f16, name="w1_sb")
            wb_sb = cp.tile([L, 256], f16, name="wb_sb")
            w3e0_sb = cp.tile([L, 33], f16, name="w3e0_sb")
            w3pp_sb = cp.tile([L, 33], f16, name="w3pp_sb")
            w3pr_sb = cp.tile([L, 33], f16, name="w3pr_sb")
            wall_sb = cp.tile([L, NBLK], f16, name="wall_sb")
            zr32 = cp.tile([1, 32], f16, name="zr32")
            s_a = cp.tile([L, BS * NBLK], f16, name="s_a")
            s_b = cp.tile([L, BS * NBLK], f16, name="s_b")
            gst = cp.tile([NPAIR, 256], f32, name="gst")
            acst = cp.tile([NPAIR, 256], f32, name="acst")
            f_sb = cp.tile([NPAIR, 256], f32, name="f_sb")

            nc.sync.dma_start(x_sb[:], x_d.ap())
            nc.sync.dma_start(w1_sb[:], w1_d.ap())
            nc.sync.dma_start(wb_sb[:], wb_d.ap())
            nc.sync.dma_start(w3e0_sb[:], w3e0_d.ap())
            nc.sync.dma_start(w3pp_sb[:], w3pp_d.ap())
            nc.sync.dma_start(w3pr_sb[:], w3pr_d.ap())
            nc.sync.dma_start(wall_sb[:], wall_d.ap())
            nc.vector.memset(zr32[:], 0.0)
            nc.vector.memset(acst[:], A256)

            loop_ctx = (tc.For_i(0, dyn_rep, 1) if dyn_rep > 1
                        else contextlib.nullcontext())
            with loop_ctx:
              for rep in range(1):

                def s1_round(ci, rnd):
                    """S1 matmuls for channel ci, batches 4*rnd..4*rnd+3,
                    followed by one fused sigmoid into s tile."""
                    s_cur = (s_a, s_b)[ci % 2]
                    nb = nb_u[ci]
                    ps = ps1p.tile([L, 2048], f32, name=f"ps1_{ci}_{rnd}",
                                   tag="s1")
                    for b in range(nb):
                        wap = w1_sb[:, (woff[ci] + b) * L:
                                    (woff[ci] + b + 1) * L]
                        for i in range(4):
                            bs = 4 * rnd + i
                            nc.tensor.matmul(
                                ps[:, i * 512 + b: i * 512 + NBLK], wap,
                                x_sb[:, bs * NBLK: bs * NBLK + NBLK - b],
                                start=(b == 0), stop=(b == nb - 1))
                    src = ap3(ps[:], 0, [[512, 4], [1, NBLK]])
                    nc.scalar.activation(
                        s_cur[:, rnd * 2000: rnd * 2000 + 2000], src,
                        mybir.ActivationFunctionType.Sigmoid)

                pf_box = [None]

                def s2_pair(p, bs, d4s):
                    """S2 + relu + S3 for pair (channel p, batch bs).

                    S3: each pair contributes one frame row to the current
                    32-pair psum group pf32 [32, 250]; the row is selected
                    by leading zero columns in the stationaries.
                    """
                    ps = ps2p.tile([L, 512], f32, name=f"ps2_{p}_{bs}",
                                   tag="s2")
                    d4 = d4s[bs // 4]
                    dsl = d4[:, (bs % 4) * NBLK: (bs % 4) * NBLK + NBLK]
                    nc.tensor.matmul(ps[:, 0:NBLK], wb_sb[:, 0:L], dsl,
                                     start=True, stop=False)
                    nc.tensor.matmul(ps[:, 1:NBLK], wb_sb[:, L:2 * L],
                                     dsl[:, 0:NBLK - 1],
                                     start=False, stop=True)
                    y4 = wp.tile([L, NBLK], f16,
                                 name=f"y4_{p}_{bs}", tag=f"y4{bs % 4}")
                    nc.vector.scalar_tensor_tensor(
                        y4[:], ps[:, 0:NBLK], 0.0,
                        wall_sb[:], mybir.AluOpType.max,
                        mybir.AluOpType.mult)
                    if stage != 'full':
                        nc.vector.tensor_scalar_add(
                            gst[0:128, bs * 4: bs * 4 + 4],
                            y4[0:128, 0:4], 0.0)
                        return
                    row = (p - 1) * 8 + bs
                    g, m = row // 32, row % 32
                    if m == 0:
                        pf_box[0] = ps3p.tile([32, 256], f32,
                                              name=f"pf32_{g}", tag="pf")
                        nc.tensor.matmul(pf_box[0][0:32, 0:NFRM],
                                         zr32[0:1, 0:32],
                                         x_sb[0:1, 0:NFRM],
                                         start=True, stop=False)
                    pf = pf_box[0]
                    last = (m == 31)
                    nc.tensor.matmul(pf[0:m + 1, 0:NFRM],
                                     w3e0_sb[:, 32 - m:33],
                                     y4[:, 0:2 * NFRM:2],
                                     start=False, stop=False)
                    nc.tensor.matmul(pf[0:m + 1, 1:NFRM],
                                     w3pp_sb[:, 32 - m:33],
                                     y4[:, 0:2 * NFRM - 2:2],
                                     start=False, stop=False)
                    nc.tensor.matmul(pf[0:m + 1, 1:NFRM],
                                     w3pr_sb[:, 32 - m:33],
                                     y4[:, 1:2 * NFRM - 1:2],
                                     start=False, stop=last)
                    if last:
                        nc.vector.tensor_scalar_add(
                            gst[32 * g: 32 * g + 32, 0:NFRM],
                            pf[0:32, 0:NFRM], 0.0)

                # iteration ci: S1+sigmoid for channel ci; S2+S3 for pair
                # p=ci-1 (whose s tiles were finished in earlier iterations).
                # The S2/S3 matmuls sit between S1 rounds so the PE never
                # waits for the Act engine to drain ps1.
                for ci in range(CPC + 2):
                    p = ci - 1
                    d4s = []
                    if p >= 1 and stage != 's1':
                        s_cur = (s_a, s_b)[p % 2]
                        s_prev = (s_a, s_b)[(p + 1) % 2]
                        for rnd in range(2):
                            d4 = dp.tile([L, 2000], f16,
                                         name=f"d4_{p}_{rnd}", tag=f"d{rnd}")
                            nc.vector.tensor_sub(
                                d4[:],
                                s_cur[:, rnd * 2000: rnd * 2000 + 2000],
                                s_prev[:, rnd * 2000: rnd * 2000 + 2000])
                            d4s.append(d4)
                    for rnd in range(2):
                        if ci <= CPC:
                            s1_round(ci, rnd)
                        if p >= 1 and stage not in ('s1',):
                            for bs in range(4 * rnd, 4 * rnd + 4):
                                s2_pair(p, bs, d4s)
                if stage != 'full':
                    # consume s tiles so nothing is dead-code eliminated
                    nc.vector.tensor_scalar_add(gst[0:128, 0:4], s_a[:, 0:4], 0.0)
                    nc.vector.tensor_scalar_add(gst[0:128, 4:8], s_b[:, 0:4], 0.0)
                if stage == 'full':
                    nc.vector.tensor_tensor_scan(
                        f_sb[:, 0:NFRM], acst[:, 0:NFRM], gst[:, 0:NFRM],
                        0.0, mybir.AluOpType.mult, mybir.AluOpType.add)
                    nc.sync.dma_start(out_d.ap(), f_sb[:, 0:NFRM])
                else:
                    nc.sync.dma_start(out_d.ap(), gst[:, 0:NFRM])
    _dedupe_ldweights(nc)
    nc.compile()
    return nc


def _dedupe_ldweights(nc):
    """Drop PE weight loads whose stationary operand matches the previous
    load in the scheduled PE stream (the splitter emits one per matmul)."""
    from concourse import mybir
    dropped = 0
    for bb in nc.m.functions[0].blocks:
        last_key = None
        keep = []
        for inst in bb.instructions:
            if isinstance(inst, mybir.InstLdweights):
                si = inst.sync_info
                key = str(inst.ins[0])
                if (key == last_key and not (si and (si.on_wait or si.on_update))):
                    dropped += 1
                    continue
                last_key = key
            elif isinstance(inst, (mybir.InstUnconditionalBranch,
                                   mybir.InstCompareAndBranch)):
                last_key = None
            keep.append(inst)
        if len(keep) != len(bb.instructions):
            bb.instructions = keep
    return dropped


def _make_runner(nc):
    """Persistent jitted 8-core runner (mirrors bass2jax.run_bass_via_pjrt)."""
    import jax
    from jax.sharding import Mesh, PartitionSpec
    from jax.experimental.shard_map import shard_map
    from concourse import bass2jax, mybir

    bass2jax.install_neuronx_cc_hook()

    partition_name = (
        nc.partition_id_tensor.name if nc.partition_id_tensor else None
    )
    in_names, out_names, out_avals, zero_shapes = [], [], [], []
    for alloc in nc.m.functions[0].allocations:
        if not isinstance(alloc, mybir.MemoryLocationSet):
            continue
        name = alloc.memorylocations[0].name
        if alloc.kind == "ExternalInput":
            if name != partition_name:
                in_names.append(name)
        elif alloc.kind == "ExternalOutput":
            out_names.append(name)
            shape = tuple(alloc.tensor_shape)
            dtype = mybir.dt.np(alloc.dtype)
            out_avals.append(jax.core.ShapedArray(shape, dtype))
            zero_shapes.append((shape, dtype))
    n_params = len(in_names)
    all_in_names = list(in_names) + list(out_names)
    if partition_name is not None:
        all_in_names.append(partition_name)

    def _body(*args):
        operands = list(args)
        if partition_name is not None:
            operands.append(bass2jax.partition_id_tensor())
        outs = bass2jax._bass_exec_p.bind(
            *operands,
            out_avals=tuple(out_avals),
            in_names=tuple(all_in_names),
            out_names=tuple(out_names),
            lowering_input_output_aliases=(),
            sim_require_finite=True,
            sim_require_nnan=True,
            nc=nc,
        )
        return tuple(outs)

    devices = jax.devices()[:NCORE]
    mesh = Mesh(np.asarray(devices), ("core",))
    n_outs = len(out_names)
    sharded = jax.jit(
        shard_map(_body, mesh=mesh,
                  in_specs=(PartitionSpec("core"),) * (n_params + n_outs),
                  out_specs=(PartitionSpec("core"),) * n_outs,
                  check_rep=False),
        donate_argnums=tuple(range(n_params, n_params + n_outs)),
        keep_unused=True,
    )

    def run(in_maps):
        concat_in = [
            np.concatenate([np.asarray(m[name]) for m in in_maps], axis=0)
            for name in in_names
        ]
        concat_zeros = [
            np.zeros((NCORE * s[0], *s[1:]), d) for (s, d) in zero_shapes
        ]
        out_arrs = sharded(*concat_in, *concat_zeros)
        return [
            {name: np.asarray(out_arrs[i]).reshape(NCORE, *out_avals[i].shape)[c]
             for i, name in enumerate(out_names)}
            for c in range(NCORE)
        ]

    return run


def _get_runner(wavData, coch_B, coch_A):
    in_maps, nb_u = _host_prep(wavData, coch_B, coch_A)
    if nb_u not in _cache:
        nc = _build(nb_u)
        _cache[nb_u] = _make_runner(nc)
    return _cache[nb_u], in_maps


def kernel(wavData, coch_B, coch_A):
    run, in_maps = _get_runner(wavData, coch_B, coch_A)
    results = run(in_maps)
    out = np.zeros((BS, NCH, NFRM), np.float32)
    for k in range(NCORE):
        F = results[k]["out"]                      # [128 pairs, 250]
        out[:, CPC * k + 1: CPC * (k + 1) + 1, :] = \
            F.reshape(CPC, BS, NFRM).transpose(1, 0, 2)
    return out


# revision 15
# speedup vs baseline: 1.4594x; 1.0229x over previous
"""Auditory spectrogram kernel for Trainium2 (8 NeuronCores, Bass/Tile).

Pipeline per the reference:
  y1 = order-4 IIR cochlear filterbank (129 channels, per-channel B/A) over wav [8, 64000]
  y2 = sigmoid(y1); y2 = 1st-order IIR (beta) over time
  y4 = relu(y2[c] - y2[c-1]); y5 = 1st-order IIR (alpha); downsample every 256 -> [8, 129, 250]

All linear recurrences are blocked-FIR matmuls on TensorE (fp16 operands,
fp32 PSUM). Channel 0's output is exactly zero, so the 128 real output
channels are sharded 16 per core; each core computes 16 channels + a
1-channel halo.

Layout: time blocked into 500 blocks of 128; partition = position in block,
free = (batch, block). Per core:
  S1  per-channel banded-Toeplitz matmuls -> psum [128, 4x512] (4 batches)
  Act one fused sigmoid per 4 batches -> s tiles fp16
  S2  hair-cell LPF: T0/T1 Toeplitz matmuls on d = s_cur - s_prev
  DVE relu with folded alpha-weights: y4w = max(psum,0) * w_all -> y4 tiles
  S3  temporal integration: frame-chunk stationaries accumulate
      PF[frame-in-chunk, (Fchunk, pair)] on PE; PE-transpose to
      [pair, frame]; frame-rate scan (alpha^256) on DVE; DMA out.
"""

import numpy as np

NCH, BS, T = 129, 8, 64000
L = 128                      # time block
NBLK = T // L                # 500 blocks
NFRM = 250                   # output frames (stride 256)
NCORE = 8
CPC = 16                     # output channels per core
NPAIR = 128                  # (channel, batch) pairs per core
BETA = float(np.exp(-1.0 / 8.0))
ALPHA = float(np.exp(-1.0 / 128.0))
A256 = float(ALPHA ** 256)
KMAX = 1024
TAIL_TOL = 4e-3

_cache = {}


def _impulse_responses(coch_B, coch_A):
    """h[c, k] for k < KMAX, float64, from the order-4 IIR coefficients."""
    B = coch_B.astype(np.float64)
    A = coch_A.astype(np.float64)
    h = np.zeros((NCH, KMAX))
    for t in range(KMAX):
        acc = B[:, t].copy() if t < 5 else np.zeros(NCH)
        for k in range(1, 5):
            if t - k >= 0:
                acc -= A[:, k] * h[:, t - k]
        h[:, t] = acc
    return h


def _band_matrix(hc, b):
    """T_b[p_in, p_out] = h[128*b + p_out - p_in] (0 where the tap index < 0)."""
    p = np.arange(L)
    idx = 128 * b + p[None, :] - p[:, None]
    valid = idx >= 0
    out = np.where(valid, hc[np.clip(idx, 0, KMAX - 1)], 0.0)
    return out


def _host_prep(wavData, coch_B, coch_A):
    wavData = np.asarray(wavData, dtype=np.float32)
    coch_B = np.asarray(coch_B, dtype=np.float64)
    coch_A = np.asarray(coch_A, dtype=np.float64)
    h = _impulse_responses(coch_B, coch_A)
    tails = np.cumsum(np.abs(h[:, ::-1]), axis=1)[:, ::-1]
    taps = np.array([
        int(np.argmax(tails[c] < TAIL_TOL)) if tails[c, 0] >= TAIL_TOL else 1
        for c in range(NCH)
    ])
    nb = np.clip(np.ceil(taps / 128.0).astype(int), 2, 8)
    # SPMD: every core runs the same program, so band counts must be uniform
    # per local channel position (max across cores).
    nb_u = [max(int(nb[CPC * k + i]) for k in range(NCORE)) for i in range(CPC + 1)]
    nbtot = sum(nb_u)
    woff = np.cumsum([0] + nb_u)

    # x: [128 pos, (bs, block)] fp16, same for all cores
    x16 = np.ascontiguousarray(
        wavData.reshape(BS, NBLK, L).transpose(2, 0, 1).reshape(L, BS * NBLK)
    ).astype(np.float16)

    w1s = []
    for k in range(NCORE):
        W1 = np.zeros((L, nbtot * L), np.float16)
        for i in range(CPC + 1):
            c = CPC * k + i
            for b in range(nb_u[i]):
                W1[:, (woff[i] + b) * L:(woff[i] + b + 1) * L] = \
                    _band_matrix(h[c], b).astype(np.float16)
        w1s.append(W1)

    p = np.arange(L)
    T0 = np.where(p[None, :] >= p[:, None], BETA ** (p[None, :] - p[:, None]), 0.0)
    T1 = np.where(p[:, None] > p[None, :], BETA ** (128 + p[None, :] - p[:, None]), 0.0)
    WB = np.concatenate([T0, T1], axis=1).astype(np.float16)

    # S3 stationaries. Matmul outputs must start at partition 0, so the
    # target row m inside the 16-pair psum group is selected by leading
    # zero columns: slice W[:, 16-m : 17] = [zeros*m | profile].
    # (alpha weights are folded into y4 by the relu, so profiles are 0/1.)
    W3E0 = np.zeros((L, 33), np.float16)
    W3E0[0, 32] = 1.0         # e0: the frame sample itself
    W3PP = np.zeros((L, 33), np.float16)
    W3PP[1:, 32] = 1.0        # prev-prev block, p>=1
    W3PR = np.zeros((L, 33), np.float16)
    W3PR[:, 32] = 1.0         # prev block

    # relu weight fold: even blocks get the prev-prev profile alpha^(256-p)
    # except p=0 which serves e0 (weight 1); odd blocks get alpha^(128-p).
    WALL = np.zeros((L, NBLK), np.float32)
    WALL[:, 0::2] = (ALPHA ** (256.0 - p))[:, None]
    WALL[0, 0::2] = 1.0
    WALL[:, 1::2] = (ALPHA ** (128.0 - p))[:, None]
    WALL = WALL.astype(np.float16)

    ins = [dict(x=x16, w1=w1s[k], wb=WB, w3e0=W3E0, w3pp=W3PP, w3pr=W3PR,
                wall=WALL)
           for k in range(NCORE)]
    return ins, tuple(nb_u)


def _build(nb_u, dyn_rep=1, stage='full'):
    import contextlib
    import concourse.bacc as bacc
    import concourse.tile as tile
    from concourse import mybir
    from concourse.ap import AP

    nbtot = sum(nb_u)
    woff = np.cumsum([0] + list(nb_u))
    f16, f32 = mybir.dt.float16, mybir.dt.float32

    nc = bacc.Bacc("TRN2", target_bir_lowering=False, debug=False,
                   num_devices=NCORE)
    x_d = nc.dram_tensor("x", [L, BS * NBLK], f16, kind="ExternalInput")
    w1_d = nc.dram_tensor("w1", [L, nbtot * L], f16, kind="ExternalInput")
    wb_d = nc.dram_tensor("wb", [L, 256], f16, kind="ExternalInput")
    w3e0_d = nc.dram_tensor("w3e0", [L, 33], f16, kind="ExternalInput")
    w3pp_d = nc.dram_tensor("w3pp", [L, 33], f16, kind="ExternalInput")
    w3pr_d = nc.dram_tensor("w3pr", [L, 33], f16, kind="ExternalInput")
    wall_d = nc.dram_tensor("wall", [L, NBLK], f16, kind="ExternalInput")
    out_d = nc.dram_tensor("out", [NPAIR, NFRM], f32, kind="ExternalOutput")

    def ap3(base_ap, off, dims):
        return AP(tensor=base_ap.tensor, offset=off,
                  ap=[list(base_ap.ap[0])] + [list(d) for d in dims])

    with tile.TileContext(nc) as tc:
        with tc.tile_pool(name="const", bufs=1) as cp, \
             tc.tile_pool(name="dp", bufs=2) as dp, \
             tc.tile_pool(name="wp", bufs=4) as wp, \
             tc.tile_pool(name="ps1", bufs=1, space="PSUM") as ps1p, \
             tc.tile_pool(name="ps2", bufs=3, space="PSUM") as ps2p, \
             tc.tile_pool(name="ps3", bufs=1, space="PSUM") as ps3p:
            x_sb = cp.tile([L, BS * NBLK], f16, name="x_sb")
            w1_sb = cp.tile([L, nbtot * L], f16, name="w1_sb")
            wb_sb = cp.tile([L, 256], f16, name="wb_sb")
            w3e0_sb = cp.tile([L, 33], f16, name="w3e0_sb")
            w3pp_sb = cp.tile([L, 33], f16, name="w3pp_sb")
            w3pr_sb = cp.tile([L, 33], f16, name="w3pr_sb")
            wall_sb = cp.tile([L, NBLK], f16, name="wall_sb")
            zr32 = cp.tile([1, 32], f16, name="zr32")
            s_a = cp.tile([L, BS * NBLK], f16, name="s_a")
            s_b = cp.tile([L, BS * NBLK], f16, name="s_b")
            gst = cp.tile([NPAIR, 256], f32, name="gst")
            acst = cp.tile([NPAIR, 256], f32, name="acst")
            f_sb = cp.tile([NPAIR, 256], f32, name="f_sb")

            nc.sync.dma_start(x_sb[:], x_d.ap())
            nc.sync.dma_start(w1_sb[:], w1_d.ap())
            nc.sync.dma_start(wb_sb[:], wb_d.ap())
            nc.sync.dma_start(w3e0_sb[:], w3e0_d.ap())
            nc.sync.dma_start(w3pp_sb[:], w3pp_d.ap())
            nc.sync.dma_start(w3pr_sb[:], w3pr_d.ap())
            nc.sync.dma_start(wall_sb[:], wall_d.ap())
            nc.vector.memset(zr32[:], 0.0)
            nc.vector.memset(acst[:], A256)

            loop_ctx = (tc.For_i(0, dyn_rep, 1) if dyn_rep > 1
                        else contextlib.nullcontext())
            with loop_ctx:
              for rep in range(1):

                def s1_round(ci, rnd):
                    """S1 matmuls for channel ci, batches 4*rnd..4*rnd+3,
                    followed by one fused sigmoid into s tile."""
                    s_cur = (s_a, s_b)[ci % 2]
                    nb = nb_u[ci]
                    ps = ps1p.tile([L, 2048], f32, name=f"ps1_{ci}_{rnd}",
                                   tag="s1")
                    for b in range(nb):
                        wap = w1_sb[:, (woff[ci] + b) * L:
                                    (woff[ci] + b + 1) * L]
                        for i in range(4):
                            bs = 4 * rnd + i
                            nc.tensor.matmul(
                                ps[:, i * 512 + b: i * 512 + NBLK], wap,
                                x_sb[:, bs * NBLK: bs * NBLK + NBLK - b],
                                start=(b == 0), stop=(b == nb - 1))
                    src = ap3(ps[:], 0, [[512, 4], [1, NBLK]])
                    nc.scalar.activation(
                        s_cur[:, rnd * 2000: rnd * 2000 + 2000], src,
                        mybir.ActivationFunctionType.Sigmoid)

                pf_box = [None]

                s3_q = []

                def s2_pair(p, bs, d4s):
                    """S2 + relu for pair (channel p, batch bs); queue S3."""
                    ps = ps2p.tile([L, 512], f32, name=f"ps2_{p}_{bs}",
                                   tag="s2")
                    d4 = d4s[bs // 4]
                    dsl = d4[:, (bs % 4) * NBLK: (bs % 4) * NBLK + NBLK]
                    nc.tensor.matmul(ps[:, 0:NBLK], wb_sb[:, 0:L], dsl,
                                     start=True, stop=False)
                    nc.tensor.matmul(ps[:, 1:NBLK], wb_sb[:, L:2 * L],
                                     dsl[:, 0:NBLK - 1],
                                     start=False, stop=True)
                    y4 = wp.tile([L, NBLK], f16,
                                 name=f"y4_{p}_{bs}", tag=f"y4{bs % 4}")
                    nc.vector.scalar_tensor_tensor(
                        y4[:], ps[:, 0:NBLK], 0.0,
                        wall_sb[:], mybir.AluOpType.max,
                        mybir.AluOpType.mult)
                    if stage != 'full':
                        nc.vector.tensor_scalar_add(
                            gst[0:128, bs * 4: bs * 4 + 4],
                            y4[0:128, 0:4], 0.0)
                        return
                    s3_q.append(((p - 1) * 8 + bs, y4))

                def s3_drain(keep=1):
                    """S3 for queued pairs: one frame row per pair in the
                    current 32-pair psum group pf32 [32, 250]; the row is
                    selected by leading zero columns in the stationaries.
                    Emitted one pair behind S2 so the PE never waits on the
                    relu (DVE) of the pair it just produced."""
                    while len(s3_q) > keep:
                        row, y4 = s3_q.pop(0)
                        g, m = row // 32, row % 32
                        if m == 0:
                            pf_box[0] = ps3p.tile([32, 256], f32,
                                                  name=f"pf32_{g}", tag="pf")
                            nc.tensor.matmul(pf_box[0][0:32, 0:NFRM],
                                             zr32[0:1, 0:32],
                                             x_sb[0:1, 0:NFRM],
                                             start=True, stop=False)
                        pf = pf_box[0]
                        last = (m == 31)
                        nc.tensor.matmul(pf[0:m + 1, 0:NFRM],
                                         w3e0_sb[:, 32 - m:33],
                                         y4[:, 0:2 * NFRM:2],
                                         start=False, stop=False)
                        nc.tensor.matmul(pf[0:m + 1, 1:NFRM],
                                         w3pp_sb[:, 32 - m:33],
                                         y4[:, 0:2 * NFRM - 2:2],
                                         start=False, stop=False)
                        nc.tensor.matmul(pf[0:m + 1, 1:NFRM],
                                         w3pr_sb[:, 32 - m:33],
                                         y4[:, 1:2 * NFRM - 1:2],
                                         start=False, stop=last)
                        if last:
                            nc.vector.tensor_scalar_add(
                                gst[32 * g: 32 * g + 32, 0:NFRM],
                                pf[0:32, 0:NFRM], 0.0)

                # iteration ci: S1+sigmoid for channel ci; S2+S3 for pair
                # p=ci-1 (whose s tiles were finished in earlier iterations).
                # The S2/S3 matmuls sit between S1 rounds so the PE never
                # waits for the Act engine to drain ps1.
                for ci in range(CPC + 2):
                    p = ci - 1
                    d4s = []
                    if p >= 1 and stage != 's1':
                        s_cur = (s_a, s_b)[p % 2]
                        s_prev = (s_a, s_b)[(p + 1) % 2]
                        for rnd in range(2):
                            d4 = dp.tile([L, 2000], f16,
                                         name=f"d4_{p}_{rnd}", tag=f"d{rnd}")
                            nc.vector.tensor_sub(
                                d4[:],
                                s_cur[:, rnd * 2000: rnd * 2000 + 2000],
                                s_prev[:, rnd * 2000: rnd * 2000 + 2000])
                            d4s.append(d4)
                    for rnd in range(2):
                        if ci <= CPC:
                            s1_round(ci, rnd)
                        if p >= 1 and stage not in ('s1',):
                            for bs in range(4 * rnd, 4 * rnd + 4):
                                s2_pair(p, bs, d4s)
                                if stage == 'full':
                                    s3_drain(keep=1)
                if stage == 'full':
                    s3_drain(keep=0)
                if stage != 'full':
                    # consume s tiles so nothing is dead-code eliminated
                    nc.vector.tensor_scalar_add(gst[0:128, 0:4], s_a[:, 0:4], 0.0)
                    nc.vector.tensor_scalar_add(gst[0:128, 4:8], s_b[:, 0:4], 0.0)
                if stage == 'full':
                    nc.vector.tensor_tensor_scan(
                        f_sb[:, 0:NFRM], acst[:, 0:NFRM], gst[:, 0:NFRM],
                        0.0, mybir.AluOpType.mult, mybir.AluOpType.add)
                    nc.sync.dma_start(out_d.ap(), f_sb[:, 0:NFRM])
                else:
                    nc.sync.dma_start(out_d.ap(), gst[:, 0:NFRM])
    _dedupe_ldweights(nc)
    nc.compile()
    return nc


def _dedupe_ldweights(nc):
    """Drop PE weight loads whose stationary operand matches the previous
    load in the scheduled PE stream (the splitter emits one per matmul)."""
    from concourse import mybir
    dropped = 0
    for bb in nc.m.functions[0].blocks:
        last_key = None
        keep = []
        for inst in bb.instructions:
            if isinstance(inst, mybir.InstLdweights):
                si = inst.sync_info
                key = str(inst.ins[0])
                if (key == last_key and not (si and (si.on_wait or si.on_update))):
                    dropped += 1
                    continue
                last_key = key
            elif isinstance(inst, (mybir.InstUnconditionalBranch,
                                   mybir.InstCompareAndBranch)):
                last_key = None
            keep.append(inst)
        if len(keep) != len(bb.instructions):
            bb.instructions = keep
    return dropped


def _make_runner(nc):
    """Persistent jitted 8-core runner (mirrors bass2jax.run_bass_via_pjrt)."""
    import jax
    from jax.sharding import Mesh, PartitionSpec
    from jax.experimental.shard_map import shard_map
    from concourse import bass2jax, mybir

    bass2jax.install_neuronx_cc_hook()

    partition_name = (
        nc.partition_id_tensor.name if nc.partition_id_tensor else None
    )
    in_names, out_names, out_avals, zero_shapes = [], [], [], []
    for alloc in nc.m.functions[0].allocations:
        if not isinstance(alloc, mybir.MemoryLocationSet):
            continue
        name = alloc.memorylocations[0].name
        if alloc.kind == "ExternalInput":
            if name != partition_name:
                in_names.append(name)
        elif alloc.kind == "ExternalOutput":
            out_names.append(name)
            shape = tuple(alloc.tensor_shape)
            dtype = mybir.dt.np(alloc.dtype)
            out_avals.append(jax.core.ShapedArray(shape, dtype))
            zero_shapes.append((shape, dtype))
    n_params = len(in_names)
    all_in_names = list(in_names) + list(out_names)
    if partition_name is not None:
        all_in_names.append(partition_name)

    def _body(*args):
        operands = list(args)
        if partition_name is not None:
            operands.append(bass2jax.partition_id_tensor())
        outs = bass2jax._bass_exec_p.bind(
            *operands,
            out_avals=tuple(out_avals),
            in_names=tuple(all_in_names),
            out_names=tuple(out_names),
            lowering_input_output_aliases=(),
            sim_require_finite=True,
            sim_require_nnan=True,
            nc=nc,
        )
        return tuple(outs)

    devices = jax.devices()[:NCORE]
    mesh = Mesh(np.asarray(devices), ("core",))
    n_outs = len(out_names)
    sharded = jax.jit(
        shard_map(_body, mesh=mesh,
                  in_specs=(PartitionSpec("core"),) * (n_params + n_outs),
                  out_specs=(PartitionSpec("core"),) * n_outs,
                  check_rep=False),
        donate_argnums=tuple(range(n_params, n_params + n_outs)),
        keep_unused=True,
    )

    def run(in_maps):
        concat_in = [
            np.concatenate([np.asarray(m[name]) for m in in_maps], axis=0)
            for name in in_names
        ]
        concat_zeros = [
            np.zeros((NCORE * s[0], *s[1:]), d) for (s, d) in zero_shapes
        ]
        out_arrs = sharded(*concat_in, *concat_zeros)
        return [
            {name: np.asarray(out_arrs[i]).reshape(NCORE, *out_avals[i].shape)[c]
             for i, name in enumerate(out_names)}
            for c in range(NCORE)
        ]

    return run


def _get_runner(wavData, coch_B, coch_A):
    in_maps, nb_u = _host_prep(wavData, coch_B, coch_A)
    if nb_u not in _cache:
        nc = _build(nb_u)
        _cache[nb_u] = _make_runner(nc)
    return _cache[nb_u], in_maps


def kernel(wavData, coch_B, coch_A):
    run, in_maps = _get_runner(wavData, coch_B, coch_A)
    results = run(in_maps)
    out = np.zeros((BS, NCH, NFRM), np.float32)
    for k in range(NCORE):
        F = results[k]["out"]                      # [128 pairs, 250]
        out[:, CPC * k + 1: CPC * (k + 1) + 1, :] = \
            F.reshape(CPC, BS, NFRM).transpose(1, 0, 2)
    return out


# revision 17
# speedup vs baseline: 1.7181x; 1.1772x over previous
"""Auditory spectrogram kernel for Trainium2 (8 NeuronCores, Bass/Tile).

Pipeline per the reference:
  y1 = order-4 IIR cochlear filterbank (129 channels, per-channel B/A) over wav [8, 64000]
  y2 = sigmoid(y1); y2 = 1st-order IIR (beta) over time
  y4 = relu(y2[c] - y2[c-1]); y5 = 1st-order IIR (alpha); downsample every 256 -> [8, 129, 250]

All linear recurrences are blocked-FIR matmuls on TensorE (fp16 operands,
fp32 PSUM). Channel 0's output is exactly zero, so the 128 real output
channels are sharded 16 per core; each core computes 16 channels + a
1-channel halo.

Layout: time blocked into 500 blocks of 128; partition = position in block,
free = (batch, block). Per core:
  S1  per-channel banded-Toeplitz matmuls -> psum [128, 4x512] (4 batches)
  Act one fused sigmoid per 4 batches -> s tiles fp16
  S2  hair-cell LPF: T0/T1 Toeplitz matmuls on d = s_cur - s_prev
  DVE relu with folded alpha-weights: y4w = max(psum,0) * w_all -> y4 tiles
  S3  temporal integration: frame-chunk stationaries accumulate
      PF[frame-in-chunk, (Fchunk, pair)] on PE; PE-transpose to
      [pair, frame]; frame-rate scan (alpha^256) on DVE; DMA out.
"""

import numpy as np

NCH, BS, T = 129, 8, 64000
L = 128                      # time block
NBLK = T // L                # 500 blocks
NFRM = 250                   # output frames (stride 256)
NCORE = 8
CPC = 16                     # output channels per core
NPAIR = 128                  # (channel, batch) pairs per core
BETA = float(np.exp(-1.0 / 8.0))
ALPHA = float(np.exp(-1.0 / 128.0))
A256 = float(ALPHA ** 256)
KMAX = 1024
TAIL_TOL = 4e-3

_cache = {}


def _impulse_responses(coch_B, coch_A):
    """h[c, k] for k < KMAX, float64, from the order-4 IIR coefficients."""
    B = coch_B.astype(np.float64)
    A = coch_A.astype(np.float64)
    h = np.zeros((NCH, KMAX))
    for t in range(KMAX):
        acc = B[:, t].copy() if t < 5 else np.zeros(NCH)
        for k in range(1, 5):
            if t - k >= 0:
                acc -= A[:, k] * h[:, t - k]
        h[:, t] = acc
    return h


def _band_matrix(hc, b):
    """T_b[p_in, p_out] = h[128*b + p_out - p_in] (0 where the tap index < 0)."""
    p = np.arange(L)
    idx = 128 * b + p[None, :] - p[:, None]
    valid = idx >= 0
    out = np.where(valid, hc[np.clip(idx, 0, KMAX - 1)], 0.0)
    return out


def _host_prep(wavData, coch_B, coch_A):
    wavData = np.asarray(wavData, dtype=np.float32)
    coch_B = np.asarray(coch_B, dtype=np.float64)
    coch_A = np.asarray(coch_A, dtype=np.float64)
    h = _impulse_responses(coch_B, coch_A)
    tails = np.cumsum(np.abs(h[:, ::-1]), axis=1)[:, ::-1]
    taps = np.array([
        int(np.argmax(tails[c] < TAIL_TOL)) if tails[c, 0] >= TAIL_TOL else 1
        for c in range(NCH)
    ])
    nb = np.clip(np.ceil(taps / 128.0).astype(int), 2, 8)
    # SPMD: every core runs the same program, so band counts must be uniform
    # per local channel position (max across cores).
    nb_u = [max(int(nb[CPC * k + i]) for k in range(NCORE)) for i in range(CPC + 1)]
    nbtot = sum(nb_u)
    woff = np.cumsum([0] + nb_u)

    # x: [128 pos, (bs, block)] fp16, same for all cores
    x16 = np.ascontiguousarray(
        wavData.reshape(BS, NBLK, L).transpose(2, 0, 1).reshape(L, BS * NBLK)
    ).astype(np.float16)

    w1s = []
    for k in range(NCORE):
        W1 = np.zeros((L, nbtot * L), np.float16)
        for i in range(CPC + 1):
            c = CPC * k + i
            for b in range(nb_u[i]):
                W1[:, (woff[i] + b) * L:(woff[i] + b + 1) * L] = \
                    _band_matrix(h[c], b).astype(np.float16)
        w1s.append(W1)

    import ml_dtypes
    p = np.arange(L)
    T0 = np.where(p[None, :] >= p[:, None], BETA ** (p[None, :] - p[:, None]), 0.0)
    T1 = np.where(p[:, None] > p[None, :], BETA ** (128 + p[None, :] - p[:, None]), 0.0)
    # S2 via fp8 DoubleRow: d = 0.5*(tanh(y1_c/2) - tanh(y1_p/2)) exactly
    # equals sigmoid(y1_c) - sigmoid(y1_p); the +/- sits in the k-tile pair.
    f8t = ml_dtypes.float8_e4m3
    W0h = (0.5 * T0).astype(f8t)
    W1h = (0.5 * T1).astype(f8t)
    WB8A = np.concatenate([-W0h, W0h], axis=1)          # ktile0=-, ktile1=+
    WB8B = np.concatenate([-W1h, W1h], axis=1)

    # S3 stationaries. Matmul outputs must start at partition 0, so the
    # target row m inside the 16-pair psum group is selected by leading
    # zero columns: slice W[:, 16-m : 17] = [zeros*m | profile].
    # (alpha weights are folded into y4 by the relu, so profiles are 0/1.)
    W3E0 = np.zeros((L, 33), np.float16)
    W3E0[0, 32] = 1.0         # e0: the frame sample itself
    W3PP = np.zeros((L, 33), np.float16)
    W3PP[1:, 32] = 1.0        # prev-prev block, p>=1
    W3PR = np.zeros((L, 33), np.float16)
    W3PR[:, 32] = 1.0         # prev block

    # relu weight fold: even blocks get the prev-prev profile alpha^(256-p)
    # except p=0 which serves e0 (weight 1); odd blocks get alpha^(128-p).
    WALL = np.zeros((L, NBLK), np.float32)
    WALL[:, 0::2] = (ALPHA ** (256.0 - p))[:, None]
    WALL[0, 0::2] = 1.0
    WALL[:, 1::2] = (ALPHA ** (128.0 - p))[:, None]
    WALL = WALL.astype(np.float16)

    ins = [dict(x=x16, w1=w1s[k], wb8a=WB8A, wb8b=WB8B, w3e0=W3E0,
                w3pp=W3PP, w3pr=W3PR, wall=WALL)
           for k in range(NCORE)]
    return ins, tuple(nb_u)


def _build(nb_u, dyn_rep=1, stage='full'):
    import contextlib
    import concourse.bacc as bacc
    import concourse.tile as tile
    from concourse import mybir
    from concourse.ap import AP

    nbtot = sum(nb_u)
    woff = np.cumsum([0] + list(nb_u))
    f16, f32 = mybir.dt.float16, mybir.dt.float32
    f8 = mybir.dt.float8e4
    DR = mybir.MatmulPerfMode.DoubleRow
    TCH = 8 * 501

    nc = bacc.Bacc("TRN2", target_bir_lowering=False, debug=False,
                   num_devices=NCORE)
    x_d = nc.dram_tensor("x", [L, BS * NBLK], f16, kind="ExternalInput")
    w1_d = nc.dram_tensor("w1", [L, nbtot * L], f16, kind="ExternalInput")
    wb8a_d = nc.dram_tensor("wb8a", [L, 256], f8, kind="ExternalInput")
    wb8b_d = nc.dram_tensor("wb8b", [L, 256], f8, kind="ExternalInput")
    w3e0_d = nc.dram_tensor("w3e0", [L, 33], f16, kind="ExternalInput")
    w3pp_d = nc.dram_tensor("w3pp", [L, 33], f16, kind="ExternalInput")
    w3pr_d = nc.dram_tensor("w3pr", [L, 33], f16, kind="ExternalInput")
    wall_d = nc.dram_tensor("wall", [L, NBLK], f16, kind="ExternalInput")
    out_d = nc.dram_tensor("out", [NPAIR, NFRM], f32, kind="ExternalOutput")

    def ap3(base_ap, off, dims):
        return AP(tensor=base_ap.tensor, offset=off,
                  ap=[list(base_ap.ap[0])] + [list(d) for d in dims])

    with tile.TileContext(nc) as tc:
        with tc.tile_pool(name="const", bufs=1) as cp, \
             tc.tile_pool(name="dp", bufs=2) as dp, \
             tc.tile_pool(name="wp", bufs=4) as wp, \
             tc.tile_pool(name="ps1", bufs=1, space="PSUM") as ps1p, \
             tc.tile_pool(name="ps2", bufs=3, space="PSUM") as ps2p, \
             tc.tile_pool(name="ps3", bufs=1, space="PSUM") as ps3p:
            x_sb = cp.tile([L, BS * NBLK], f16, name="x_sb")
            w1_sb = cp.tile([L, nbtot * L], f16, name="w1_sb")
            wb8a_sb = cp.tile([L, 256], f8, name="wb8a_sb")
            wb8b_sb = cp.tile([L, 256], f8, name="wb8b_sb")
            w3e0_sb = cp.tile([L, 33], f16, name="w3e0_sb")
            w3pp_sb = cp.tile([L, 33], f16, name="w3pp_sb")
            w3pr_sb = cp.tile([L, 33], f16, name="w3pr_sb")
            wall_sb = cp.tile([L, NBLK], f16, name="wall_sb")
            zr32 = cp.tile([1, 32], f16, name="zr32")
            t_all = cp.tile([L, (CPC + 1) * TCH], f8, name="t_all")
            gst = cp.tile([NPAIR, 256], f32, name="gst")
            acst = cp.tile([NPAIR, 256], f32, name="acst")
            f_sb = cp.tile([NPAIR, 256], f32, name="f_sb")

            nc.sync.dma_start(x_sb[:], x_d.ap())
            nc.sync.dma_start(w1_sb[:], w1_d.ap())
            nc.sync.dma_start(wb8a_sb[:], wb8a_d.ap())
            nc.sync.dma_start(wb8b_sb[:], wb8b_d.ap())
            nc.sync.dma_start(w3e0_sb[:], w3e0_d.ap())
            nc.sync.dma_start(w3pp_sb[:], w3pp_d.ap())
            nc.sync.dma_start(w3pr_sb[:], w3pr_d.ap())
            nc.sync.dma_start(wall_sb[:], wall_d.ap())
            nc.vector.memset(zr32[:], 0.0)
            nc.vector.memset(t_all[:, 0:34068], 0.0)
            nc.vector.memset(t_all[:, 34068:(CPC + 1) * TCH], 0.0)
            nc.vector.memset(acst[:], A256)

            loop_ctx = (tc.For_i(0, dyn_rep, 1) if dyn_rep > 1
                        else contextlib.nullcontext())
            with loop_ctx:
              for rep in range(1):

                def s1_round(ci, rnd):
                    """S1 matmuls for channel ci, batches 4*rnd..4*rnd+3,
                    followed by one fused sigmoid into s tile."""
                    nb = nb_u[ci]
                    ps = ps1p.tile([L, 2048], f32, name=f"ps1_{ci}_{rnd}",
                                   tag="s1")
                    for b in range(nb):
                        wap = w1_sb[:, (woff[ci] + b) * L:
                                    (woff[ci] + b + 1) * L]
                        for i in range(4):
                            bs = 4 * rnd + i
                            nc.tensor.matmul(
                                ps[:, i * 512 + b: i * 512 + NBLK], wap,
                                x_sb[:, bs * NBLK: bs * NBLK + NBLK - b],
                                start=(b == 0), stop=(b == nb - 1))
                    src = ap3(ps[:], 0, [[512, 4], [1, NBLK]])
                    dst = ap3(t_all[:], ci * TCH + rnd * 4 * 501 + 1,
                              [[501, 4], [1, NBLK]])
                    nc.scalar.activation(
                        dst, src, mybir.ActivationFunctionType.Tanh,
                        scale=0.5)

                pf_box = [None]

                s3_q = []

                def s2_pair(p, bs):
                    """S2 + relu for pair (channel p, batch bs); queue S3.
                    DoubleRow fp8: ktile0 = t[p-1] with -W, ktile1 = t[p]
                    with +W, so the channel diff happens inside the PE."""
                    ps = ps2p.tile([L, 512], f32, name=f"ps2_{p}_{bs}",
                                   tag="s2")
                    base = (p - 1) * TCH + bs * 501 + 1
                    lwa = ap3(wb8a_sb[:], 0, [[128, 2], [1, 128]])
                    lwb = ap3(wb8b_sb[:], 0, [[128, 2], [1, 128]])
                    rh0 = ap3(t_all[:], base, [[TCH, 2], [1, NBLK]])
                    rh1 = ap3(t_all[:], base - 1, [[TCH, 2], [1, NBLK]])
                    nc.tensor.matmul(ps[:, 0:NBLK], lwa, rh0,
                                     start=True, stop=False, perf_mode=DR)
                    nc.tensor.matmul(ps[:, 0:NBLK], lwb, rh1,
                                     start=False, stop=True, perf_mode=DR)
                    y4 = wp.tile([L, NBLK], f16,
                                 name=f"y4_{p}_{bs}", tag=f"y4{bs % 4}")
                    nc.vector.scalar_tensor_tensor(
                        y4[:], ps[:, 0:NBLK], 0.0,
                        wall_sb[:], mybir.AluOpType.max,
                        mybir.AluOpType.mult)
                    if stage != 'full':
                        nc.vector.tensor_scalar_add(
                            gst[0:128, bs * 4: bs * 4 + 4],
                            y4[0:128, 0:4], 0.0)
                        return
                    s3_q.append(((p - 1) * 8 + bs, y4))

                def s3_drain(keep=1):
                    """S3 for queued pairs: one frame row per pair in the
                    current 32-pair psum group pf32 [32, 250]; the row is
                    selected by leading zero columns in the stationaries.
                    Emitted one pair behind S2 so the PE never waits on the
                    relu (DVE) of the pair it just produced."""
                    while len(s3_q) > keep:
                        row, y4 = s3_q.pop(0)
                        g, m = row // 32, row % 32
                        if m == 0:
                            pf_box[0] = ps3p.tile([32, 256], f32,
                                                  name=f"pf32_{g}", tag="pf")
                            nc.tensor.matmul(pf_box[0][0:32, 0:NFRM],
                                             zr32[0:1, 0:32],
                                             x_sb[0:1, 0:NFRM],
                                             start=True, stop=False)
                        pf = pf_box[0]
                        last = (m == 31)
                        nc.tensor.matmul(pf[0:m + 1, 0:NFRM],
                                         w3e0_sb[:, 32 - m:33],
                                         y4[:, 0:2 * NFRM:2],
                                         start=False, stop=False)
                        nc.tensor.matmul(pf[0:m + 1, 1:NFRM],
                                         w3pp_sb[:, 32 - m:33],
                                         y4[:, 0:2 * NFRM - 2:2],
                                         start=False, stop=False)
                        nc.tensor.matmul(pf[0:m + 1, 1:NFRM],
                                         w3pr_sb[:, 32 - m:33],
                                         y4[:, 1:2 * NFRM - 1:2],
                                         start=False, stop=last)
                        if last:
                            nc.vector.tensor_scalar_add(
                                gst[32 * g: 32 * g + 32, 0:NFRM],
                                pf[0:32, 0:NFRM], 0.0)

                # iteration ci: S1+sigmoid for channel ci; S2+S3 for pair
                # p=ci-1 (whose s tiles were finished in earlier iterations).
                # The S2/S3 matmuls sit between S1 rounds so the PE never
                # waits for the Act engine to drain ps1.
                for ci in range(CPC + 2):
                    p = ci - 1
                    for rnd in range(2):
                        if ci <= CPC:
                            s1_round(ci, rnd)
                        if p >= 1 and stage not in ('s1',):
                            for bs in range(4 * rnd, 4 * rnd + 4):
                                s2_pair(p, bs)
                                if stage == 'full':
                                    s3_drain(keep=1)
                if stage == 'full':
                    s3_drain(keep=0)
                if stage != 'full':
                    # consume t_all so nothing is dead-code eliminated
                    nc.vector.tensor_scalar_add(gst[0:128, 0:4],
                                                t_all[:, 0:4], 0.0)
                if stage == 'full':
                    nc.vector.tensor_tensor_scan(
                        f_sb[:, 0:NFRM], acst[:, 0:NFRM], gst[:, 0:NFRM],
                        0.0, mybir.AluOpType.mult, mybir.AluOpType.add)
                    nc.sync.dma_start(out_d.ap(), f_sb[:, 0:NFRM])
                else:
                    nc.sync.dma_start(out_d.ap(), gst[:, 0:NFRM])
    _dedupe_ldweights(nc)
    nc.compile()
    return nc


def _dedupe_ldweights(nc):
    """Drop PE weight loads whose stationary operand matches the previous
    load in the scheduled PE stream (the splitter emits one per matmul)."""
    from concourse import mybir
    dropped = 0
    for bb in nc.m.functions[0].blocks:
        last_key = None
        keep = []
        for inst in bb.instructions:
            if isinstance(inst, mybir.InstLdweights):
                si = inst.sync_info
                key = str(inst.ins[0])
                if (key == last_key and not (si and (si.on_wait or si.on_update))):
                    dropped += 1
                    continue
                last_key = key
            elif isinstance(inst, (mybir.InstUnconditionalBranch,
                                   mybir.InstCompareAndBranch)):
                last_key = None
            keep.append(inst)
        if len(keep) != len(bb.instructions):
            bb.instructions = keep
    return dropped


def _make_runner(nc):
    """Persistent jitted 8-core runner (mirrors bass2jax.run_bass_via_pjrt)."""
    import jax
    from jax.sharding import Mesh, PartitionSpec
    from jax.experimental.shard_map import shard_map
    from concourse import bass2jax, mybir

    bass2jax.install_neuronx_cc_hook()

    partition_name = (
        nc.partition_id_tensor.name if nc.partition_id_tensor else None
    )
    in_names, out_names, out_avals, zero_shapes = [], [], [], []
    for alloc in nc.m.functions[0].allocations:
        if not isinstance(alloc, mybir.MemoryLocationSet):
            continue
        name = alloc.memorylocations[0].name
        if alloc.kind == "ExternalInput":
            if name != partition_name:
                in_names.append(name)
        elif alloc.kind == "ExternalOutput":
            out_names.append(name)
            shape = tuple(alloc.tensor_shape)
            dtype = mybir.dt.np(alloc.dtype)
            out_avals.append(jax.core.ShapedArray(shape, dtype))
            zero_shapes.append((shape, dtype))
    n_params = len(in_names)
    all_in_names = list(in_names) + list(out_names)
    if partition_name is not None:
        all_in_names.append(partition_name)

    def _body(*args):
        operands = list(args)
        if partition_name is not None:
            operands.append(bass2jax.partition_id_tensor())
        outs = bass2jax._bass_exec_p.bind(
            *operands,
            out_avals=tuple(out_avals),
            in_names=tuple(all_in_names),
            out_names=tuple(out_names),
            lowering_input_output_aliases=(),
            sim_require_finite=True,
            sim_require_nnan=True,
            nc=nc,
        )
        return tuple(outs)

    devices = jax.devices()[:NCORE]
    mesh = Mesh(np.asarray(devices), ("core",))
    n_outs = len(out_names)
    sharded = jax.jit(
        shard_map(_body, mesh=mesh,
                  in_specs=(PartitionSpec("core"),) * (n_params + n_outs),
                  out_specs=(PartitionSpec("core"),) * n_outs,
                  check_rep=False),
        donate_argnums=tuple(range(n_params, n_params + n_outs)),
        keep_unused=True,
    )

    def run(in_maps):
        concat_in = [
            np.concatenate([np.asarray(m[name]) for m in in_maps], axis=0)
            for name in in_names
        ]
        concat_zeros = [
            np.zeros((NCORE * s[0], *s[1:]), d) for (s, d) in zero_shapes
        ]
        out_arrs = sharded(*concat_in, *concat_zeros)
        return [
            {name: np.asarray(out_arrs[i]).reshape(NCORE, *out_avals[i].shape)[c]
             for i, name in enumerate(out_names)}
            for c in range(NCORE)
        ]

    return run


def _get_runner(wavData, coch_B, coch_A):
    in_maps, nb_u = _host_prep(wavData, coch_B, coch_A)
    if nb_u not in _cache:
        nc = _build(nb_u)
        _cache[nb_u] = _make_runner(nc)
    return _cache[nb_u], in_maps


def kernel(wavData, coch_B, coch_A):
    run, in_maps = _get_runner(wavData, coch_B, coch_A)
    results = run(in_maps)
    out = np.zeros((BS, NCH, NFRM), np.float32)
    for k in range(NCORE):
        F = results[k]["out"]                      # [128 pairs, 250]
        out[:, CPC * k + 1: CPC * (k + 1) + 1, :] = \
            F.reshape(CPC, BS, NFRM).transpose(1, 0, 2)
    return out
